# revision 1
# baseline (speedup 1.0000x reference)
"""Trainium2 Bass kernel for HSEGNNFlexLayer (GNN message passing).

Strategy (8 NeuronCores, SPMD, zero collectives):
  - Host assigns each node to a (core, window, slot) bin: 8 cores x 25
    windows x 256 slots.  Every edge is routed to the core that owns its
    dst node, so the segment-sum is fully local to each core.
  - Per core, edges are grouped by window and padded to a uniform tile
    grid (NWIN x T_B x 128) so one Bass program serves all 8 cores.
  - Message layers: c = a @ Wflat computed with edges on PSUM partitions
    (lhsT = transposed, host-gathered features), attr-weighted k-sum via
    per-partition scalar_tensor_tensor chains, Silu on ScalarE.
  - Scatter-add: one-hot S matmul (lhsT=m2, rhs=S) accumulating into a
    per-window PSUM bank; flushed to an SBUF-resident transposed
    aggregate.
  - Node update layers run the same pipeline over the 6400 node slots.
"""

import numpy as np
import ml_dtypes

import concourse.bass as bass
import concourse.mybir as mybir
import concourse.tile as tile
from concourse import bacc
from concourse import bass_utils
from concourse.masks import make_identity

# Problem constants (hardcoded per contest contract)
N, E, D, A, AM = 50000, 500000, 128, 8, 3
MIN_DIM = 2 * D + AM  # 259
UIN_DIM = D + D + AM  # 259
NCORES = 8
P = 128
KO = A * D  # 1024 = flattened (k, o) output columns per TP layer
SLOTS = 256  # node slots per window (one PSUM bank of f32)
NWIN = 25
NODE_SLOTS = NWIN * SLOTS  # 6400 per core
BF16 = mybir.dt.bfloat16
F32 = mybir.dt.float32
NPBF16 = ml_dtypes.bfloat16

_cache = {}


# --------------------------------------------------------------------------
# Host-side preparation
# --------------------------------------------------------------------------

def _assign_nodes(dst):
    """Greedy-pack nodes into NCORES*NWIN bins (<=SLOTS nodes each),
    balancing per-bin edge counts.  Returns (node2bin, node2slot)."""
    import heapq

    counts = np.bincount(dst, minlength=N)
    order = np.argsort(-counts, kind="stable")
    nbins = NCORES * NWIN
    node2bin = np.empty(N, dtype=np.int32)
    node2slot = np.empty(N, dtype=np.int32)
    bin_nodes = np.zeros(nbins, dtype=np.int32)
    # heap of (edge_count, bin)
    heap = [(0, b) for b in range(nbins)]
    heapq.heapify(heap)
    pending = []
    for n in order:
        while True:
            c, b = heapq.heappop(heap)
            if bin_nodes[b] < SLOTS:
                break
            pending.append((c, b))  # full bin: drop permanently
        node2bin[n] = b
        node2slot[n] = bin_nodes[b]
        bin_nodes[b] += 1
        heapq.heappush(heap, (c + int(counts[n]), b))
    return node2bin, node2slot


def _prepare(x, edge_attr, node_attr, amf, anf, W1, b1, W2, b2, W3, b3, W4, b4,
             edge_index):
    x = np.asarray(x, dtype=np.float32)
    edge_attr = np.asarray(edge_attr, dtype=np.float32)
    node_attr = np.asarray(node_attr, dtype=np.float32)
    amf = np.asarray(amf, dtype=np.float32)
    anf = np.asarray(anf, dtype=np.float32)
    src = np.asarray(edge_index[0], dtype=np.int64).astype(np.int32)
    dst = np.asarray(edge_index[1], dtype=np.int64).astype(np.int32)

    node2bin, node2slot = _assign_nodes(dst)
    node_core = node2bin // NWIN
    node_win = node2bin % NWIN
    node_gslot = node_win * SLOTS + node2slot  # slot within core [0, NODE_SLOTS)

    e_bin = node2bin[dst]  # bin (core*NWIN + win) of each edge

    # order edges by bin
    e_order = np.argsort(e_bin, kind="stable")
    e_bin_sorted = e_bin[e_order]
    bin_cnt = np.bincount(e_bin_sorted, minlength=NCORES * NWIN)
    # tiles per window: uniform across all bins
    T_B = int(np.ceil(bin_cnt.max() / P))
    win_cap = T_B * P
    E_pad = NWIN * win_cap

    bin_starts = np.zeros(NCORES * NWIN + 1, dtype=np.int64)
    np.cumsum(bin_cnt, out=bin_starts[1:])

    # Destination position of each (sorted) edge inside its core's padded list
    offs_in_bin = np.arange(len(e_order)) - bin_starts[e_bin_sorted]
    pos = (e_bin_sorted % NWIN) * win_cap + offs_in_bin  # position within core
    core_of_edge = e_bin_sorted // NWIN

    # Per-core packed index arrays (padded entries use sentinel -1)
    ew_src = np.full((NCORES, E_pad), -1, dtype=np.int64)
    ew_dst = np.full((NCORES, E_pad), -1, dtype=np.int64)
    ew_eid = np.full((NCORES, E_pad), -1, dtype=np.int64)
    ew_src[core_of_edge, pos] = src[e_order]
    ew_dst[core_of_edge, pos] = dst[e_order]
    ew_eid[core_of_edge, pos] = e_order

    # Flattened weights (k-major columns): Wf[i, k*D + o] = W[i, k, o]
    w1f = np.ascontiguousarray(np.asarray(W1, np.float32).reshape(MIN_DIM, KO)).astype(NPBF16)
    w2f = np.ascontiguousarray(np.asarray(W2, np.float32).reshape(D, KO)).astype(NPBF16)
    w3f = np.ascontiguousarray(np.asarray(W3, np.float32).reshape(UIN_DIM, KO)).astype(NPBF16)
    w4f = np.ascontiguousarray(np.asarray(W4, np.float32).reshape(D, KO)).astype(NPBF16)
    biases = [np.ascontiguousarray(np.tile(np.asarray(b, np.float32)[None, :], (P, 1)))
              for b in (b1, b2, b3, b4)]

    xT_all = x.T.astype(NPBF16)  # [D, N]

    in_maps = []
    slot2node = np.full((NCORES, NODE_SLOTS), -1, dtype=np.int64)
    for c in range(NCORES):
        s = ew_src[c]
        d = ew_dst[c]
        eid = ew_eid[c]
        valid = eid >= 0
        sv = np.where(valid, s, 0)
        dv = np.where(valid, d, 0)
        ev = np.where(valid, eid, 0)

        xiT = xT_all[:, dv].copy()
        xjT = xT_all[:, sv].copy()
        xiT[:, ~valid] = 0
        xjT[:, ~valid] = 0
        amfT = amf[ev].T.astype(NPBF16)
        amfT[:, ~valid] = 0
        battr = edge_attr[ev].astype(np.float32)
        battr[~valid] = 0

        # scatter one-hot: local slot within window
        S = np.zeros((E_pad, SLOTS), dtype=NPBF16)
        rows = np.nonzero(valid)[0]
        S[rows, node2slot[d[rows]]] = 1

        # node side
        nodes_c = np.nonzero(node_core == c)[0]
        gs = node_gslot[nodes_c]
        slot2node[c, gs] = nodes_c
        nxT = np.zeros((D, NODE_SLOTS), dtype=NPBF16)
        nxT[:, gs] = xT_all[:, nodes_c]
        nanfT = np.zeros((AM, NODE_SLOTS), dtype=NPBF16)
        nanfT[:, gs] = anf[nodes_c].T.astype(NPBF16)
        nattr = np.zeros((NODE_SLOTS, A), dtype=np.float32)
        nattr[gs] = node_attr[nodes_c]

        in_maps.append({
            "xiT": np.ascontiguousarray(xiT),
            "xjT": np.ascontiguousarray(xjT),
            "amfT": np.ascontiguousarray(amfT),
            "battr": np.ascontiguousarray(battr),
            "S": S,
            "xT": nxT,
            "anfT": nanfT,
            "nattr": nattr,
            "w1f": w1f, "w2f": w2f, "w3f": w3f, "w4f": w4f,
            "b1r": biases[0], "b2r": biases[1], "b3r": biases[2], "b4r": biases[3],
        })
    return in_maps, slot2node, T_B, E_pad


# --------------------------------------------------------------------------
# Device kernel builder
# --------------------------------------------------------------------------

def _build(T_B, E_pad):
    nc = bacc.Bacc("TRN2", target_bir_lowering=False, debug=False,
                   num_devices=NCORES)

    d_xiT = nc.dram_tensor("xiT", [D, E_pad], BF16, kind="ExternalInput")
    d_xjT = nc.dram_tensor("xjT", [D, E_pad], BF16, kind="ExternalInput")
    d_amfT = nc.dram_tensor("amfT", [AM, E_pad], BF16, kind="ExternalInput")
    d_battr = nc.dram_tensor("battr", [E_pad, A], F32, kind="ExternalInput")
    d_S = nc.dram_tensor("S", [E_pad, SLOTS], BF16, kind="ExternalInput")
    d_xT = nc.dram_tensor("xT", [D, NODE_SLOTS], BF16, kind="ExternalInput")
    d_anfT = nc.dram_tensor("anfT", [AM, NODE_SLOTS], BF16, kind="ExternalInput")
    d_nattr = nc.dram_tensor("nattr", [NODE_SLOTS, A], F32, kind="ExternalInput")
    d_w1f = nc.dram_tensor("w1f", [MIN_DIM, KO], BF16, kind="ExternalInput")
    d_w2f = nc.dram_tensor("w2f", [D, KO], BF16, kind="ExternalInput")
    d_w3f = nc.dram_tensor("w3f", [UIN_DIM, KO], BF16, kind="ExternalInput")
    d_w4f = nc.dram_tensor("w4f", [D, KO], BF16, kind="ExternalInput")
    d_b = [nc.dram_tensor(f"b{i}r", [P, D], F32, kind="ExternalInput")
           for i in (1, 2, 3, 4)]
    d_out = nc.dram_tensor("out", [NODE_SLOTS, D], F32, kind="ExternalOutput")

    mult = mybir.AluOpType.mult
    add = mybir.AluOpType.add
    silu = mybir.ActivationFunctionType.Silu

    with tile.TileContext(nc) as tc:
        with (
            tc.tile_pool(name="const", bufs=1) as cpool,
            tc.tile_pool(name="ain", bufs=3) as apool,
            tc.tile_pool(name="work", bufs=3) as wpool,
            tc.tile_pool(name="cps", bufs=2, space="PSUM") as cps,
            tc.tile_pool(name="trps", bufs=2, space="PSUM") as trps,
            tc.tile_pool(name="aggps", bufs=1, space="PSUM") as aggps,
        ):
            # ---- constants resident in SBUF ----
            ident = cpool.tile([P, P], BF16, tag="ident", name="ident")
            make_identity(nc, ident[:])

            w1c = [cpool.tile([P, KO], BF16, tag="w1c0", name="w1c0"),
                   cpool.tile([P, KO], BF16, tag="w1c1", name="w1c1"),
                   cpool.tile([AM, KO], BF16, tag="w1c2", name="w1c2")]
            nc.sync.dma_start(w1c[0][:], d_w1f.ap()[0:P, :])
            nc.sync.dma_start(w1c[1][:], d_w1f.ap()[P:2 * P, :])
            nc.sync.dma_start(w1c[2][:], d_w1f.ap()[2 * P:MIN_DIM, :])
            w2c = cpool.tile([P, KO], BF16, tag="w2c", name="w2c")
            nc.sync.dma_start(w2c[:], d_w2f.ap())
            w3c = [cpool.tile([P, KO], BF16, tag="w3c0", name="w3c0"),
                   cpool.tile([P, KO], BF16, tag="w3c1", name="w3c1"),
                   cpool.tile([AM, KO], BF16, tag="w3c2", name="w3c2")]
            nc.sync.dma_start(w3c[0][:], d_w3f.ap()[0:P, :])
            nc.sync.dma_start(w3c[1][:], d_w3f.ap()[P:2 * P, :])
            nc.sync.dma_start(w3c[2][:], d_w3f.ap()[2 * P:UIN_DIM, :])
            w4c = cpool.tile([P, KO], BF16, tag="w4c", name="w4c")
            nc.sync.dma_start(w4c[:], d_w4f.ap())

            btile = [cpool.tile([P, D], F32, tag=f"b{i}r", name=f"b{i}r")
                     for i in range(4)]
            for i in range(4):
                nc.sync.dma_start(btile[i][:], d_b[i].ap())

            aggT = cpool.tile([P, NODE_SLOTS], BF16, tag="aggT", name="aggT")

            # ---- helper: one TP layer tile (c = lhs-chunks @ wflat,
            #      weighted k-sum + bias, optional silu) ----
            def tp_layer(chunks, wchunks, bt, bias_rep, out_tile, do_silu):
                cpsum = cps.tile([P, KO], F32, tag="c", name="c")
                nch = len(chunks)
                for ci in range(nch):
                    for h in range(2):
                        nc.tensor.matmul(
                            cpsum[:, h * 512:(h + 1) * 512],
                            lhsT=chunks[ci],
                            rhs=wchunks[ci][:, h * 512:(h + 1) * 512],
                            start=(ci == 0),
                            stop=(ci == nch - 1),
                        )
                acc = wpool.tile([P, D], F32, tag="acc", name="acc")
                nc.vector.scalar_tensor_tensor(
                    acc[:], cpsum[:, 0:D], bt[:, 0:1], bias_rep[:], mult, add)
                for k in range(1, A):
                    nc.vector.scalar_tensor_tensor(
                        acc[:], cpsum[:, k * D:(k + 1) * D], bt[:, k:k + 1],
                        acc[:], mult, add)
                if do_silu:
                    nc.scalar.activation(out_tile[:], acc[:], silu)
                else:
                    nc.vector.tensor_copy(out_tile[:], acc[:])

            def transpose_to(src_bf16):
                tps = trps.tile([P, P], BF16, tag="tr", name="tr")
                nc.tensor.transpose(tps[:], src_bf16[:], ident[:])
                dst = wpool.tile([P, P], BF16, tag="mT", name="mT")
                nc.vector.tensor_copy(dst[:], tps[:])
                return dst

            # ---- edge phase ----
            GT = 4  # tiles fetched per DMA group
            agg_hold = [None]
            ntiles = NWIN * T_B
            for g0 in range(0, ntiles, GT):
                gn = min(GT, ntiles - g0)
                e0 = g0 * P
                ew = gn * P
                xi4 = apool.tile([P, GT * P], BF16, tag="xi4", name="xi4")
                xj4 = apool.tile([P, GT * P], BF16, tag="xj4", name="xj4")
                am4 = apool.tile([AM, GT * P], BF16, tag="am4", name="am4")
                nc.sync.dma_start(xi4[:, :ew], d_xiT.ap()[:, e0:e0 + ew])
                nc.sync.dma_start(xj4[:, :ew], d_xjT.ap()[:, e0:e0 + ew])
                nc.sync.dma_start(am4[:, :ew], d_amfT.ap()[:, e0:e0 + ew])
                for j in range(gn):
                    t = g0 + j
                    w = t // T_B
                    tw = t % T_B
                    bt = apool.tile([P, A], F32, tag="bt", name="bt")
                    nc.sync.dma_start(
                        bt[:], d_battr.ap()[t * P:(t + 1) * P, :])
                    St = apool.tile([P, SLOTS], BF16, tag="St", name="St")
                    nc.sync.dma_start(
                        St[:], d_S.ap()[t * P:(t + 1) * P, :])

                    m1 = wpool.tile([P, D], BF16, tag="m1", name="m1")
                    tp_layer([xi4[:, j * P:(j + 1) * P],
                              xj4[:, j * P:(j + 1) * P],
                              am4[:, j * P:(j + 1) * P]],
                             w1c, bt, btile[0], m1, True)
                    m1T = transpose_to(m1)
                    m2 = wpool.tile([P, D], BF16, tag="m2", name="m2")
                    tp_layer([m1T], [w2c], bt, btile[1], m2, True)

                    if tw == 0:
                        agg_hold[0] = aggps.tile([P, SLOTS], F32, tag="agg", name="agg")
                    agg_ps = agg_hold[0]
                    nc.tensor.matmul(
                        agg_ps[:],
                        lhsT=m2[:],
                        rhs=St[:],
                        start=(tw == 0),
                        stop=(tw == T_B - 1),
                    )
                    if tw == T_B - 1:
                        nc.vector.tensor_copy(
                            aggT[:, w * SLOTS:(w + 1) * SLOTS], agg_ps[:])

            # ---- node phase ----
            nnt = NODE_SLOTS // P  # 50
            for g0 in range(0, nnt, GT):
                gn = min(GT, nnt - g0)
                n0 = g0 * P
                nw = gn * P
                xt4 = apool.tile([P, GT * P], BF16, tag="xi4", name="xi4")
                an4 = apool.tile([AM, GT * P], BF16, tag="am4", name="am4")
                nc.sync.dma_start(xt4[:, :nw], d_xT.ap()[:, n0:n0 + nw])
                nc.sync.dma_start(an4[:, :nw], d_anfT.ap()[:, n0:n0 + nw])
                for j in range(gn):
                    t = g0 + j
                    na = apool.tile([P, A], F32, tag="bt", name="bt")
                    nc.sync.dma_start(
                        na[:], d_nattr.ap()[t * P:(t + 1) * P, :])
                    u = wpool.tile([P, D], BF16, tag="m1", name="m1")
                    tp_layer([xt4[:, j * P:(j + 1) * P],
                              aggT[:, t * P:(t + 1) * P],
                              an4[:, j * P:(j + 1) * P]],
                             w3c, na, btile[2], u, True)
                    uT = transpose_to(u)
                    out_t = wpool.tile([P, D], F32, tag="outt", name="outt")
                    tp_layer([uT], [w4c], na, btile[3], out_t, False)
                    nc.sync.dma_start(
                        d_out.ap()[t * P:(t + 1) * P, :], out_t[:])

    nc.compile()
    return nc


# --------------------------------------------------------------------------
# Entry point
# --------------------------------------------------------------------------

def kernel(x, edge_attr, node_attr, additional_message_features,
           additional_node_features, W1, b1, W2, b2, W3, b3, W4, b4,
           edge_index, batch=None):
    in_maps, slot2node, T_B, E_pad = _prepare(
        x, edge_attr, node_attr, additional_message_features,
        additional_node_features, W1, b1, W2, b2, W3, b3, W4, b4, edge_index)

    key = (T_B, E_pad)
    if key not in _cache:
        _cache[key] = _build(T_B, E_pad)
    nc = _cache[key]

    res = bass_utils.run_bass_kernel_spmd(
        nc, in_maps, core_ids=list(range(NCORES)))
    kernel.last = (nc, in_maps, res)

    out = np.zeros((N, D), dtype=np.float32)
    for c in range(NCORES):
        oc = res.results[c]["out"]
        mask = slot2node[c] >= 0
        out[slot2node[c][mask]] = oc[mask]
    return out



# revision 4
# speedup vs baseline: 12.3585x; 12.3585x over previous
"""Trainium2 Bass kernel for HSEGNNFlexLayer (GNN message passing).

Strategy (8 NeuronCores, SPMD):
  - Host assigns each node to a (core, window, slot) bin: 8 cores x 25
    windows x 256 slots.  Every edge is routed to the core that owns its
    dst node, so the segment-sum is fully local to each core.
  - x is staged SHARDED (one slot-major [6400, 128] bf16 shard per core,
    1.6MB each) and assembled on-device into a replicated slot-major
    table via an AllGather over NeuronLink.  x_i / x_j are then gathered
    ON DEVICE with dma_gather(transpose=True), which lands feature-major
    tiles directly — the host never stages per-edge gathered features.
  - The scatter one-hot S is built on device per tile via
    tensor_scalar(is_equal) against an iota row, from a staged slot id.
  - Message layers: c = a @ Wflat with edges on PSUM partitions,
    attr-weighted k-sum via scalar_tensor_tensor chains, Silu on ScalarE.
  - Scatter-add: one-hot S matmul accumulating into a per-window PSUM
    bank; flushed to an SBUF-resident transposed aggregate.
  - Node update layers run the same pipeline over the 6400 node slots.

Execution path: a cached jax.jit of the bass_exec shard_map with
explicit (threaded) device_put of the per-call inputs — every call still
moves all inputs host->device and all outputs device->host.
"""

import numpy as np
import ml_dtypes

import concourse.bass as bass
import concourse.mybir as mybir
import concourse.tile as tile
from concourse import bacc

# Problem constants (hardcoded per contest contract)
N, E, D, A, AM = 50000, 500000, 128, 8, 3
MIN_DIM = 2 * D + AM  # 259
UIN_DIM = D + D + AM  # 259
NCORES = 8
P = 128
KO = A * D  # 1024 = flattened (k, o) output columns per TP layer
SLOTS = 256  # node slots per window (one PSUM bank of f32)
NWIN = 25
NODE_SLOTS = NWIN * SLOTS  # 6400 per core
VTOT = NCORES * NODE_SLOTS  # 51200 rows in the allgathered table
HALF = VTOT // 2
ZROW = 24 * SLOTS  # reserved always-zero slot (window 24, slot 0)
GCH = 512  # dma_gather chunk (hardware transpose-gather limit is ~896)
BF16 = mybir.dt.bfloat16
F16 = mybir.dt.float16
F32 = mybir.dt.float32
I16 = mybir.dt.int16
I32 = mybir.dt.int32
NPBF16 = ml_dtypes.bfloat16

_nc_cache = {}
_exec_cache = {}


# --------------------------------------------------------------------------
# Host-side preparation
# --------------------------------------------------------------------------

def _assign_nodes(dst):
    """Greedy-pack nodes into NCORES*NWIN bins (<=SLOTS nodes each),
    balancing per-bin edge counts.  Slot 0 of window NWIN-1 on every core
    is reserved (stays zero) so the gather tables have a known zero row.
    Returns (node2bin, node2slot)."""
    import heapq

    counts = np.bincount(dst, minlength=N)
    order = np.argsort(-counts, kind="stable")
    nbins = NCORES * NWIN
    node2bin = np.empty(N, dtype=np.int32)
    node2slot = np.empty(N, dtype=np.int32)
    bin_nodes = np.zeros(nbins, dtype=np.int32)
    for c in range(NCORES):
        bin_nodes[c * NWIN + (NWIN - 1)] = 1  # reserve the zero row
    heap = [(0, b) for b in range(nbins)]
    heapq.heapify(heap)
    for n in order:
        while True:
            c, b = heapq.heappop(heap)
            if bin_nodes[b] < SLOTS:
                break
        node2bin[n] = b
        node2slot[n] = bin_nodes[b]
        bin_nodes[b] += 1
        heapq.heappush(heap, (c + int(counts[n]), b))
    return node2bin, node2slot


def _wrap16(a):
    """Pack idx vector into the SWDGE [16, n/16] layout then replicate to
    [128, n/16] (one copy per gpsimd DSP core) is done on DEVICE; host
    returns the [16, n/16] block."""
    return np.ascontiguousarray(a.reshape(-1, 16).T).astype(np.int16)


def _prepare(x, edge_attr, node_attr, amf, anf, W1, b1, W2, b2, W3, b3, W4, b4,
             edge_index):
    x = np.asarray(x, dtype=np.float32)
    edge_attr = np.asarray(edge_attr, dtype=np.float32)
    node_attr = np.asarray(node_attr, dtype=np.float32)
    amf = np.asarray(amf, dtype=np.float32)
    anf = np.asarray(anf, dtype=np.float32)
    src = np.asarray(edge_index[0]).astype(np.int32)
    dst = np.asarray(edge_index[1]).astype(np.int32)

    node2bin, node2slot = _assign_nodes(dst)
    node_core = node2bin // NWIN
    node_win = node2bin % NWIN
    node_gslot = node_win * SLOTS + node2slot  # slot within core [0, NODE_SLOTS)

    e_bin = node2bin[dst]  # bin (core*NWIN + win) of each edge

    e_order = np.argsort(e_bin, kind="stable")
    e_bin_sorted = e_bin[e_order]
    bin_cnt = np.bincount(e_bin_sorted, minlength=NCORES * NWIN)
    T_B = int(np.ceil(bin_cnt.max() / P))
    win_cap = T_B * P
    E_pad = NWIN * win_cap
    ntiles = NWIN * T_B

    bin_starts = np.zeros(NCORES * NWIN + 1, dtype=np.int64)
    np.cumsum(bin_cnt, out=bin_starts[1:])

    offs_in_bin = np.arange(len(e_order)) - bin_starts[e_bin_sorted]
    pos = (e_bin_sorted % NWIN) * win_cap + offs_in_bin  # position within core
    core_of_edge = e_bin_sorted // NWIN

    # Per-core packed edge arrays (padded entries: eid -1)
    ew_src = np.zeros((NCORES, E_pad), dtype=np.int32)
    ew_dst = np.zeros((NCORES, E_pad), dtype=np.int32)
    ew_eid = np.full((NCORES, E_pad), -1, dtype=np.int64)
    ew_src[core_of_edge, pos] = src[e_order]
    ew_dst[core_of_edge, pos] = dst[e_order]
    ew_eid[core_of_edge, pos] = e_order

    # Flattened weights (k-major columns): Wf[i, k*D + o] = W[i, k, o]
    w1f = np.ascontiguousarray(np.asarray(W1, np.float32).reshape(MIN_DIM, KO)).astype(NPBF16)
    w2f = np.ascontiguousarray(np.asarray(W2, np.float32).reshape(D, KO)).astype(NPBF16)
    w3f = np.ascontiguousarray(np.asarray(W3, np.float32).reshape(UIN_DIM, KO)).astype(NPBF16)
    w4f = np.ascontiguousarray(np.asarray(W4, np.float32).reshape(D, KO)).astype(NPBF16)
    biases = [np.ascontiguousarray(np.tile(np.asarray(b, np.float32)[None, :], (P, 1)))
              for b in (b1, b2, b3, b4)]

    g_all = node_core.astype(np.int64) * NODE_SLOTS + node_gslot  # global slot per node

    # identity idx for the node-phase transpose-gather (same on all cores)
    nid = _wrap16(np.arange(NODE_SLOTS, dtype=np.int16))

    in_maps = []
    slot2node = np.full((NCORES, NODE_SLOTS), -1, dtype=np.int64)
    for c in range(NCORES):
        s = ew_src[c]
        d = ew_dst[c]
        eid = ew_eid[c]
        valid = eid >= 0
        ev = np.where(valid, eid, 0)

        # gather indices
        gi_dst = node_gslot[d].astype(np.int32)
        gi_dst[~valid] = ZROW
        g_src = g_all[s]
        idxa = np.where(valid & (g_src < HALF), g_src, ZROW).astype(np.int16)
        idxb = np.where(valid & (g_src >= HALF), g_src - HALF, ZROW).astype(np.int16)

        # slot of each edge within its window, [128, ntiles] (slot[p,t] for
        # edge t*128+p); -1 for padding -> one-hot row of zeros
        slotf = np.where(valid, node2slot[d].astype(np.float32), -1.0)
        slotf = np.ascontiguousarray(slotf.reshape(ntiles, P).T)

        battr = edge_attr[ev].astype(NPBF16)
        battr[~valid] = 0
        amfT = amf[ev].T.astype(NPBF16)
        amfT[:, ~valid] = 0

        # node side (slot-major shard)
        nodes_c = np.nonzero(node_core == c)[0]
        gs = node_gslot[nodes_c]
        slot2node[c, gs] = nodes_c
        xsh = np.zeros((NODE_SLOTS, D), dtype=NPBF16)
        xsh[gs] = x[nodes_c].astype(NPBF16)
        anfT = np.zeros((AM, NODE_SLOTS), dtype=NPBF16)
        anfT[:, gs] = anf[nodes_c].T.astype(NPBF16)
        nattr = np.zeros((NODE_SLOTS, A), dtype=NPBF16)
        nattr[gs] = node_attr[nodes_c].astype(NPBF16)

        in_maps.append({
            "xsh": xsh,
            "xi_i": _wrap16(gi_dst.astype(np.int16)),
            "xja_i": _wrap16(idxa),
            "xjb_i": _wrap16(idxb),
            "nid_i": nid,
            "slotf": np.ascontiguousarray(slotf),
            "battr": np.ascontiguousarray(battr),
            "amfT": np.ascontiguousarray(amfT),
            "nattr": nattr,
            "anfT": anfT,
            "w1f": w1f, "w2f": w2f, "w3f": w3f, "w4f": w4f,
            "b1r": biases[0], "b2r": biases[1], "b3r": biases[2], "b4r": biases[3],
        })
    return in_maps, slot2node, T_B, E_pad


# --------------------------------------------------------------------------
# Device kernel builder
# --------------------------------------------------------------------------

def _build(T_B):
    win_cap = T_B * P
    E_pad = NWIN * win_cap
    ntiles = NWIN * T_B
    n_gch = win_cap // GCH  # full gather chunks per window
    rem = win_cap - n_gch * GCH

    nc = bacc.Bacc("TRN2", target_bir_lowering=False, debug=False,
                   num_devices=NCORES)

    d_xsh = nc.dram_tensor("xsh", [NODE_SLOTS, D], BF16, kind="ExternalInput")
    d_xii = nc.dram_tensor("xi_i", [16, E_pad // 16], I16, kind="ExternalInput")
    d_xja = nc.dram_tensor("xja_i", [16, E_pad // 16], I16, kind="ExternalInput")
    d_xjb = nc.dram_tensor("xjb_i", [16, E_pad // 16], I16, kind="ExternalInput")
    d_nid = nc.dram_tensor("nid_i", [16, NODE_SLOTS // 16], I16, kind="ExternalInput")
    d_slot = nc.dram_tensor("slotf", [P, ntiles], F32, kind="ExternalInput")
    d_battr = nc.dram_tensor("battr", [E_pad, A], BF16, kind="ExternalInput")
    d_amfT = nc.dram_tensor("amfT", [AM, E_pad], BF16, kind="ExternalInput")
    d_nattr = nc.dram_tensor("nattr", [NODE_SLOTS, A], BF16, kind="ExternalInput")
    d_anfT = nc.dram_tensor("anfT", [AM, NODE_SLOTS], BF16, kind="ExternalInput")
    d_w1f = nc.dram_tensor("w1f", [MIN_DIM, KO], BF16, kind="ExternalInput")
    d_w2f = nc.dram_tensor("w2f", [D, KO], BF16, kind="ExternalInput")
    d_w3f = nc.dram_tensor("w3f", [UIN_DIM, KO], BF16, kind="ExternalInput")
    d_w4f = nc.dram_tensor("w4f", [D, KO], BF16, kind="ExternalInput")
    d_b = [nc.dram_tensor(f"b{i}r", [P, D], F32, kind="ExternalInput")
           for i in (1, 2, 3, 4)]
    d_out = nc.dram_tensor("out", [NODE_SLOTS, D], F16, kind="ExternalOutput")

    # raw Internal DRAM (dma_gather source must be Internal Local)
    x_loc = nc.dram_tensor("x_loc", [NODE_SLOTS, D], BF16)
    x_all = nc.dram_tensor("x_all", [VTOT, D], BF16)

    mult = mybir.AluOpType.mult
    add = mybir.AluOpType.add
    iseq = mybir.AluOpType.is_equal
    silu = mybir.ActivationFunctionType.Silu

    from concourse.masks import make_identity

    with tile.TileContext(nc) as tc:
        with (
            tc.tile_pool(name="const", bufs=1) as cpool,
            tc.tile_pool(name="gat", bufs=2) as gpool,
            tc.tile_pool(name="ain", bufs=2) as apool,
            tc.tile_pool(name="work", bufs=3) as wpool,
            tc.tile_pool(name="cps", bufs=2, space="PSUM") as cps,
            tc.tile_pool(name="trps", bufs=2, space="PSUM") as trps,
            tc.tile_pool(name="aggps", bufs=1, space="PSUM") as aggps,
        ):
            # ---- phase 0: assemble the replicated slot-major x table ----
            nc.sync.dma_start(x_loc.ap(), d_xsh.ap())
            nc.gpsimd.collective_compute(
                "AllGather", mybir.AluOpType.bypass,
                replica_groups=[list(range(NCORES))],
                ins=[x_loc.ap().opt()],
                outs=[x_all.ap().opt()],
            )

            # idx slabs, replicated 8x down the partitions for the 8 Q7 cores
            xi_s = cpool.tile([P, E_pad // 16], I16, tag="xi_s", name="xi_s")
            xja_s = cpool.tile([P, E_pad // 16], I16, tag="xja_s", name="xja_s")
            xjb_s = cpool.tile([P, E_pad // 16], I16, tag="xjb_s", name="xjb_s")
            nid_s = cpool.tile([P, NODE_SLOTS // 16], I16, tag="nid_s", name="nid_s")
            for k in range(8):
                sl = slice(16 * k, 16 * (k + 1))
                nc.sync.dma_start(xi_s[sl, :], d_xii.ap())
                nc.sync.dma_start(xja_s[sl, :], d_xja.ap())
                nc.sync.dma_start(xjb_s[sl, :], d_xjb.ap())
                nc.sync.dma_start(nid_s[sl, :], d_nid.ap())

            slot_s = cpool.tile([P, ntiles], F32, tag="slot_s", name="slot_s")
            nc.sync.dma_start(slot_s[:], d_slot.ap())

            # ---- constants resident in SBUF ----
            ident = cpool.tile([P, P], BF16, tag="ident", name="ident")
            make_identity(nc, ident[:])

            iota_i = cpool.tile([P, SLOTS], I32, tag="iota_i", name="iota_i")
            nc.gpsimd.iota(iota_i[:], pattern=[[1, SLOTS]], channel_multiplier=0)
            iota_f = cpool.tile([P, SLOTS], F32, tag="iota_f", name="iota_f")
            nc.vector.tensor_copy(iota_f[:], iota_i[:])

            w1c = [cpool.tile([P, KO], BF16, tag="w1c0", name="w1c0"),
                   cpool.tile([P, KO], BF16, tag="w1c1", name="w1c1"),
                   cpool.tile([AM, KO], BF16, tag="w1c2", name="w1c2")]
            nc.sync.dma_start(w1c[0][:], d_w1f.ap()[0:P, :])
            nc.sync.dma_start(w1c[1][:], d_w1f.ap()[P:2 * P, :])
            nc.sync.dma_start(w1c[2][:], d_w1f.ap()[2 * P:MIN_DIM, :])
            w2c = cpool.tile([P, KO], BF16, tag="w2c", name="w2c")
            nc.sync.dma_start(w2c[:], d_w2f.ap())
            w3c = [cpool.tile([P, KO], BF16, tag="w3c0", name="w3c0"),
                   cpool.tile([P, KO], BF16, tag="w3c1", name="w3c1"),
                   cpool.tile([AM, KO], BF16, tag="w3c2", name="w3c2")]
            nc.sync.dma_start(w3c[0][:], d_w3f.ap()[0:P, :])
            nc.sync.dma_start(w3c[1][:], d_w3f.ap()[P:2 * P, :])
            nc.sync.dma_start(w3c[2][:], d_w3f.ap()[2 * P:UIN_DIM, :])
            w4c = cpool.tile([P, KO], BF16, tag="w4c", name="w4c")
            nc.sync.dma_start(w4c[:], d_w4f.ap())

            btile = [cpool.tile([P, D], F32, tag=f"b{i}r", name=f"b{i}r")
                     for i in range(4)]
            for i in range(4):
                nc.sync.dma_start(btile[i][:], d_b[i].ap())

            # node-side attr slabs
            na_s = cpool.tile([P, NODE_SLOTS // P * A], F32, tag="na_s", name="na_s")
            na_bf = cpool.tile([P, NODE_SLOTS // P * A], BF16, tag="na_bf", name="na_bf")
            nc.sync.dma_start(
                na_bf[:].rearrange("p (t a) -> p t a", a=A),
                d_nattr.ap().rearrange("(t p) a -> p t a", p=P))
            nc.vector.tensor_copy(na_s[:], na_bf[:])
            anf_s = cpool.tile([AM, NODE_SLOTS], BF16, tag="anf_s", name="anf_s")
            nc.sync.dma_start(anf_s[:], d_anfT.ap())

            aggT = cpool.tile([P, NODE_SLOTS], BF16, tag="aggT", name="aggT")

            # everything below (gathers) must see the finished x_all/idx slabs
            tc.strict_bb_all_engine_barrier()

            # ---- helpers ----
            def gather_T(dst_ap, src_ap, idx_slab, i0, n):
                """transpose-gather n (<=GCH) rows of src into dst (feature-
                major [128, n]); idx are columns i0/16.. of idx_slab."""
                nc.gpsimd.dma_gather(
                    out_ap=dst_ap.rearrange("p (o f) -> p o f", o=1),
                    in_ap=src_ap,
                    idxs_ap=idx_slab[:, i0 // 16:(i0 + n) // 16],
                    num_idxs=n, num_idxs_reg=n, elem_size=P, transpose=True)

            def tp_layer(chunks, wchunks, bt, bias_rep, out_tile, do_silu):
                cpsum = cps.tile([P, KO], F32, tag="c", name="c")
                nch = len(chunks)
                for ci in range(nch):
                    for h in range(2):
                        nc.tensor.matmul(
                            cpsum[:, h * 512:(h + 1) * 512],
                            lhsT=chunks[ci],
                            rhs=wchunks[ci][:, h * 512:(h + 1) * 512],
                            start=(ci == 0),
                            stop=(ci == nch - 1),
                        )
                acc = wpool.tile([P, D], F32, tag="acc", name="acc")
                nc.vector.scalar_tensor_tensor(
                    acc[:], cpsum[:, 0:D], bt[:, 0:1], bias_rep[:], mult, add)
                for k in range(1, A):
                    nc.vector.scalar_tensor_tensor(
                        acc[:], cpsum[:, k * D:(k + 1) * D], bt[:, k:k + 1],
                        acc[:], mult, add)
                if do_silu:
                    nc.scalar.activation(out_tile[:], acc[:], silu)
                else:
                    nc.vector.tensor_copy(out_tile[:], acc[:])

            def transpose_to(src_bf16):
                tps = trps.tile([P, P], BF16, tag="tr", name="tr")
                nc.tensor.transpose(tps[:], src_bf16[:], ident[:])
                dst = wpool.tile([P, P], BF16, tag="mT", name="mT")
                nc.vector.tensor_copy(dst[:], tps[:])
                return dst

            xa_half = x_all.ap()[0:HALF, :]
            xb_half = x_all.ap()[HALF:VTOT, :]

            # ---- edge phase ----
            for w in range(NWIN):
                e0 = w * win_cap
                xiw = gpool.tile([P, win_cap], BF16, tag="xiw", name="xiw")
                xjw = gpool.tile([P, win_cap], BF16, tag="xjw", name="xjw")
                xjb = gpool.tile([P, win_cap], BF16, tag="xjb", name="xjb")
                for c0 in range(0, win_cap, GCH):
                    n = min(GCH, win_cap - c0)
                    gather_T(xiw[:, c0:c0 + n], x_loc.ap(), xi_s, e0 + c0, n)
                    gather_T(xjw[:, c0:c0 + n], xa_half, xja_s, e0 + c0, n)
                    gather_T(xjb[:, c0:c0 + n], xb_half, xjb_s, e0 + c0, n)
                nc.vector.tensor_tensor(xjw[:], xjw[:], xjb[:], add)

                amfw = apool.tile([AM, win_cap], BF16, tag="amfw", name="amfw")
                nc.sync.dma_start(amfw[:], d_amfT.ap()[:, e0:e0 + win_cap])
                btw_bf = apool.tile([P, T_B * A], BF16, tag="btwb", name="btwb")
                nc.sync.dma_start(
                    btw_bf[:].rearrange("p (t a) -> p t a", a=A),
                    d_battr.ap()[e0:e0 + win_cap, :]
                    .rearrange("(t p) a -> p t a", p=P))
                btw = apool.tile([P, T_B * A], F32, tag="btw", name="btw")
                nc.vector.tensor_copy(btw[:], btw_bf[:])

                agg_ps = aggps.tile([P, SLOTS], F32, tag="agg", name="agg")
                for t in range(T_B):
                    gt = w * T_B + t
                    bt = btw[:, t * A:(t + 1) * A]

                    m1 = wpool.tile([P, D], BF16, tag="m1", name="m1")
                    tp_layer([xiw[:, t * P:(t + 1) * P],
                              xjw[:, t * P:(t + 1) * P],
                              amfw[:, t * P:(t + 1) * P]],
                             w1c, bt, btile[0], m1, True)
                    m1T = transpose_to(m1)
                    m2 = wpool.tile([P, D], BF16, tag="m2", name="m2")
                    tp_layer([m1T], [w2c], bt, btile[1], m2, True)

                    St = wpool.tile([P, SLOTS], BF16, tag="St", name="St")
                    nc.vector.tensor_scalar(
                        St[:], iota_f[:], slot_s[:, gt:gt + 1], None, iseq)
                    nc.tensor.matmul(
                        agg_ps[:],
                        lhsT=m2[:],
                        rhs=St[:],
                        start=(t == 0),
                        stop=(t == T_B - 1),
                    )
                nc.vector.tensor_copy(
                    aggT[:, w * SLOTS:(w + 1) * SLOTS], agg_ps[:])

            # ---- node phase ----
            xT = cpool.tile([P, NODE_SLOTS], BF16, tag="xT", name="xT")
            for c0 in range(0, NODE_SLOTS, GCH):
                n = min(GCH, NODE_SLOTS - c0)
                gather_T(xT[:, c0:c0 + n], x_loc.ap(), nid_s, c0, n)

            nnt = NODE_SLOTS // P  # 50
            for t in range(nnt):
                na = na_s[:, t * A:(t + 1) * A]
                u = wpool.tile([P, D], BF16, tag="m1", name="m1")
                tp_layer([xT[:, t * P:(t + 1) * P],
                          aggT[:, t * P:(t + 1) * P],
                          anf_s[:, t * P:(t + 1) * P]],
                         w3c, na, btile[2], u, True)
                uT = transpose_to(u)
                out_t = wpool.tile([P, D], F16, tag="outt", name="outt")
                tp_layer([uT], [w4c], na, btile[3], out_t, False)
                nc.sync.dma_start(d_out.ap()[t * P:(t + 1) * P, :], out_t[:])

    nc.compile()
    return nc


# --------------------------------------------------------------------------
# Cached PJRT execution (explicit sharded device_put + cached jit)
# --------------------------------------------------------------------------

def _get_exec(nc):
    key = id(nc)
    if key in _exec_cache:
        return _exec_cache[key]

    import jax
    from jax.sharding import Mesh, PartitionSpec, NamedSharding
    from jax.experimental.shard_map import shard_map
    from concourse.bass2jax import (
        _bass_exec_p, install_neuronx_cc_hook, partition_id_tensor)

    install_neuronx_cc_hook()

    partition_name = nc.partition_id_tensor.name if nc.partition_id_tensor else None
    in_names, out_names, out_avals, zero_outs = [], [], [], []
    for alloc in nc.m.functions[0].allocations:
        if not isinstance(alloc, mybir.MemoryLocationSet):
            continue
        name = alloc.memorylocations[0].name
        if alloc.kind == "ExternalInput":
            if name != partition_name:
                in_names.append(name)
        elif alloc.kind == "ExternalOutput":
            out_names.append(name)
            shape = tuple(alloc.tensor_shape)
            dtype = mybir.dt.np(alloc.dtype)
            out_avals.append(jax.core.ShapedArray(shape, dtype))
            zero_outs.append(np.zeros((NCORES * shape[0], *shape[1:]), dtype))
    n_params = len(in_names)
    n_outs = len(out_avals)
    all_in = in_names + out_names
    if partition_name is not None:
        all_in.append(partition_name)

    def _body(*args):
        operands = list(args)
        if partition_name is not None:
            operands.append(partition_id_tensor())
        outs = _bass_exec_p.bind(
            *operands,
            out_avals=tuple(out_avals),
            in_names=tuple(all_in),
            out_names=tuple(out_names),
            lowering_input_output_aliases=(),
            sim_require_finite=True,
            sim_require_nnan=True,
            nc=nc,
        )
        return tuple(outs)

    devices = jax.devices()[:NCORES]
    mesh = Mesh(np.asarray(devices), ("core",))
    sharding = NamedSharding(mesh, PartitionSpec("core"))
    donate = tuple(range(n_params, n_params + n_outs))
    sharded = jax.jit(
        shard_map(_body, mesh=mesh,
                  in_specs=(PartitionSpec("core"),) * (n_params + n_outs),
                  out_specs=(PartitionSpec("core"),) * n_outs,
                  check_rep=False),
        donate_argnums=donate,
        keep_unused=True,
    )

    meta = (in_names, out_names, out_avals, zero_outs, sharding, sharded)
    _exec_cache[key] = meta
    return meta


def _run_fast(nc, in_maps):
    """One full execution: host concat -> device transfer -> NEFF run ->
    fetch outputs.  Returns a list of per-core {name: np.ndarray}."""
    import jax
    from concurrent.futures import ThreadPoolExecutor

    in_names, out_names, out_avals, zero_outs, sharding, sharded = _get_exec(nc)

    concat_in = [
        np.concatenate([np.asarray(m[nm]) for m in in_maps], axis=0)
        for nm in in_names
    ]

    def put(a):
        return jax.device_put(a, sharding)

    with ThreadPoolExecutor(max_workers=8) as pool:
        dev_in = list(pool.map(put, concat_in + zero_outs))
    out_arrs = sharded(*dev_in)

    with ThreadPoolExecutor(max_workers=8) as pool:
        host_out = list(pool.map(np.asarray, out_arrs))
    return [
        {name: host_out[i].reshape(NCORES, *out_avals[i].shape)[c]
         for i, name in enumerate(out_names)}
        for c in range(NCORES)
    ]


# --------------------------------------------------------------------------
# Entry point
# --------------------------------------------------------------------------

def kernel(x, edge_attr, node_attr, additional_message_features,
           additional_node_features, W1, b1, W2, b2, W3, b3, W4, b4,
           edge_index, batch=None):
    in_maps, slot2node, T_B, E_pad = _prepare(
        x, edge_attr, node_attr, additional_message_features,
        additional_node_features, W1, b1, W2, b2, W3, b3, W4, b4, edge_index)

    if T_B not in _nc_cache:
        _nc_cache[T_B] = _build(T_B)
    nc = _nc_cache[T_B]

    results = _run_fast(nc, in_maps)
    kernel.last = (nc, in_maps, results)

    out = np.zeros((N, D), dtype=np.float32)
    for c in range(NCORES):
        oc = results[c]["out"].astype(np.float32)
        mask = slot2node[c] >= 0
        out[slot2node[c][mask]] = oc[mask]
    return out


# revision 5
# speedup vs baseline: 18.2960x; 1.4804x over previous
"""Trainium2 Bass kernel for HSEGNNFlexLayer (GNN message passing).

Strategy (8 NeuronCores, SPMD):
  - Host assigns each node to a (core, window, slot) bin: 8 cores x 25
    windows x 256 slots.  Every edge is routed to the core that owns its
    dst node, so the segment-sum is fully local to each core.
  - x is staged SHARDED (one slot-major [6400, 128] bf16 shard per core)
    and assembled on-device into a replicated slot-major table via an
    AllGather over NeuronLink.  Weights are likewise sharded and
    allgathered.  x_i / x_j are gathered ON DEVICE with
    dma_gather(transpose=True), which lands feature-major tiles directly
    — the host never stages per-edge gathered features.
  - The scatter one-hot S is built on device per tile via
    tensor_scalar(is_equal) against an iota row, from a staged slot id.
  - Message layers: c = a @ Wflat with edges on PSUM partitions,
    attr-weighted k-sum via scalar_tensor_tensor chains, Silu on ScalarE.
  - Scatter-add: one-hot S matmul accumulating into a per-window PSUM
    bank; flushed to an SBUF-resident transposed aggregate.
  - Node update layers run the same pipeline over the 6400 node slots.

All per-core inputs are packed into ONE contiguous byte blob so each
call performs a single large host->device transfer (the axon tunnel has
high per-array overhead).  The donated zero output buffers are generated
on-device.  Execution goes through a cached jax.jit of the bass_exec
shard_map; every call still moves all inputs host->device and all
outputs device->host.
"""

import numpy as np
import ml_dtypes

import concourse.bass as bass
import concourse.mybir as mybir
import concourse.tile as tile
from concourse import bacc

# Problem constants (hardcoded per contest contract)
N, E, D, A, AM = 50000, 500000, 128, 8, 3
MIN_DIM = 2 * D + AM  # 259
UIN_DIM = D + D + AM  # 259
NCORES = 8
P = 128
KO = A * D  # 1024 = flattened (k, o) output columns per TP layer
SLOTS = 256  # node slots per window (one PSUM bank of f32)
NWIN = 25
NODE_SLOTS = NWIN * SLOTS  # 6400 per core
VTOT = NCORES * NODE_SLOTS  # 51200 rows in the allgathered table
HALF = VTOT // 2
ZROW = 24 * SLOTS  # reserved always-zero slot (window 24, slot 0)
GCH = 512  # dma_gather chunk (hardware transpose-gather limit is ~896)
WROWS = 2 * (MIN_DIM + D) + 2  # 776 packed weight rows (pad to 8*97)
WSH = WROWS // NCORES  # 97 weight rows staged per core
BF16 = mybir.dt.bfloat16
F16 = mybir.dt.float16
F32 = mybir.dt.float32
I16 = mybir.dt.int16
I32 = mybir.dt.int32
U8 = mybir.dt.uint8
NPBF16 = ml_dtypes.bfloat16

_nc_cache = {}
_exec_cache = {}


def _layout(T_B):
    """Byte offsets of each logical tensor inside the per-core blob."""
    win_cap = T_B * P
    E_pad = NWIN * win_cap
    ntiles = NWIN * T_B
    fields = [
        ("xsh", NODE_SLOTS * D * 2),
        ("wsh", WSH * KO * 2),
        ("battr", E_pad * A * 2),
        ("amfT", AM * E_pad * 2),
        ("slot", P * ntiles * 2),
        ("xi_i", E_pad * 2),
        ("xja_i", E_pad * 2),
        ("xjb_i", E_pad * 2),
        ("nid_i", NODE_SLOTS * 2),
        ("nattr", NODE_SLOTS * A * 2),
        ("anfT", AM * NODE_SLOTS * 2),
        ("bias", 4 * D * 4),
    ]
    offs, o = {}, 0
    for name, nbytes in fields:
        offs[name] = o
        o += (nbytes + 3) & ~3  # 4-byte align
    return offs, o


# --------------------------------------------------------------------------
# Host-side preparation
# --------------------------------------------------------------------------

def _assign_nodes(dst):
    """Greedy-pack nodes into NCORES*NWIN bins (<=SLOTS nodes each),
    balancing per-bin edge counts.  Slot 0 of window NWIN-1 on every core
    is reserved (stays zero) so the gather tables have a known zero row.
    Returns (node2bin, node2slot)."""
    import heapq

    counts = np.bincount(dst, minlength=N)
    order = np.argsort(-counts, kind="stable")
    nbins = NCORES * NWIN
    node2bin = np.empty(N, dtype=np.int32)
    node2slot = np.empty(N, dtype=np.int32)
    bin_nodes = np.zeros(nbins, dtype=np.int32)
    for c in range(NCORES):
        bin_nodes[c * NWIN + (NWIN - 1)] = 1  # reserve the zero row
    heap = [(0, b) for b in range(nbins)]
    heapq.heapify(heap)
    for n in order:
        while True:
            c, b = heapq.heappop(heap)
            if bin_nodes[b] < SLOTS:
                break
        node2bin[n] = b
        node2slot[n] = bin_nodes[b]
        bin_nodes[b] += 1
        heapq.heappush(heap, (c + int(counts[n]), b))
    return node2bin, node2slot


def _wrap16(a):
    """Pack an idx vector into the SWDGE [16, n/16] layout (replication to
    [128, n/16] happens on device)."""
    return np.ascontiguousarray(a.reshape(-1, 16).T).astype(np.int16)


def _prepare(x, edge_attr, node_attr, amf, anf, W1, b1, W2, b2, W3, b3, W4, b4,
             edge_index):
    x = np.asarray(x, dtype=np.float32)
    edge_attr = np.asarray(edge_attr, dtype=np.float32)
    node_attr = np.asarray(node_attr, dtype=np.float32)
    amf = np.asarray(amf, dtype=np.float32)
    anf = np.asarray(anf, dtype=np.float32)
    src = np.asarray(edge_index[0]).astype(np.int32)
    dst = np.asarray(edge_index[1]).astype(np.int32)

    node2bin, node2slot = _assign_nodes(dst)
    node_core = node2bin // NWIN
    node_win = node2bin % NWIN
    node_gslot = node_win * SLOTS + node2slot  # slot within core [0, NODE_SLOTS)

    e_bin = node2bin[dst]  # bin (core*NWIN + win) of each edge

    e_order = np.argsort(e_bin, kind="stable")
    e_bin_sorted = e_bin[e_order]
    bin_cnt = np.bincount(e_bin_sorted, minlength=NCORES * NWIN)
    T_B = int(np.ceil(bin_cnt.max() / P))
    win_cap = T_B * P
    E_pad = NWIN * win_cap
    ntiles = NWIN * T_B

    bin_starts = np.zeros(NCORES * NWIN + 1, dtype=np.int64)
    np.cumsum(bin_cnt, out=bin_starts[1:])

    offs_in_bin = np.arange(len(e_order)) - bin_starts[e_bin_sorted]
    pos = (e_bin_sorted % NWIN) * win_cap + offs_in_bin  # position within core
    core_of_edge = e_bin_sorted // NWIN

    # Per-core packed edge arrays (padded entries: eid -1)
    ew_src = np.zeros((NCORES, E_pad), dtype=np.int32)
    ew_dst = np.zeros((NCORES, E_pad), dtype=np.int32)
    ew_eid = np.full((NCORES, E_pad), -1, dtype=np.int64)
    ew_src[core_of_edge, pos] = src[e_order]
    ew_dst[core_of_edge, pos] = dst[e_order]
    ew_eid[core_of_edge, pos] = e_order

    # Packed weight block [WROWS, KO] bf16: w1, w2, w3, w4 stacked (k-major
    # flattened columns Wf[i, k*D + o] = W[i, k, o]); sharded across cores.
    wpack = np.zeros((WROWS, KO), dtype=NPBF16)
    r = 0
    for W in (W1, W2, W3, W4):
        Wf = np.asarray(W, np.float32).reshape(-1, KO)
        wpack[r:r + Wf.shape[0]] = Wf.astype(NPBF16)
        r += Wf.shape[0]
    biases = np.stack([np.asarray(b, np.float32) for b in (b1, b2, b3, b4)])

    g_all = node_core.astype(np.int64) * NODE_SLOTS + node_gslot

    nid = _wrap16(np.arange(NODE_SLOTS, dtype=np.int16))

    offs, blob_bytes = _layout(T_B)

    def place(blob, name, arr):
        raw = arr.tobytes()
        blob[offs[name]:offs[name] + len(raw)] = np.frombuffer(raw, np.uint8)

    in_maps = []
    slot2node = np.full((NCORES, NODE_SLOTS), -1, dtype=np.int64)
    for c in range(NCORES):
        s = ew_src[c]
        d = ew_dst[c]
        eid = ew_eid[c]
        valid = eid >= 0
        ev = np.where(valid, eid, 0)

        gi_dst = node_gslot[d].astype(np.int32)
        gi_dst[~valid] = ZROW
        g_src = g_all[s]
        idxa = np.where(valid & (g_src < HALF), g_src, ZROW).astype(np.int16)
        idxb = np.where(valid & (g_src >= HALF), g_src - HALF, ZROW).astype(np.int16)

        # slot of each edge within its window, [128, ntiles] i16; -1 padding
        sloti = np.where(valid, node2slot[d], -1).astype(np.int16)
        sloti = np.ascontiguousarray(sloti.reshape(ntiles, P).T)

        battr = edge_attr[ev].astype(NPBF16)
        battr[~valid] = 0
        amfT = amf[ev].T.astype(NPBF16)
        amfT[:, ~valid] = 0

        nodes_c = np.nonzero(node_core == c)[0]
        gs = node_gslot[nodes_c]
        slot2node[c, gs] = nodes_c
        xsh = np.zeros((NODE_SLOTS, D), dtype=NPBF16)
        xsh[gs] = x[nodes_c].astype(NPBF16)
        anfT = np.zeros((AM, NODE_SLOTS), dtype=NPBF16)
        anfT[:, gs] = anf[nodes_c].T.astype(NPBF16)
        nattr = np.zeros((NODE_SLOTS, A), dtype=NPBF16)
        nattr[gs] = node_attr[nodes_c].astype(NPBF16)

        blob = np.zeros(blob_bytes, dtype=np.uint8)
        place(blob, "xsh", xsh)
        place(blob, "wsh", np.ascontiguousarray(wpack[c * WSH:(c + 1) * WSH]))
        place(blob, "battr", np.ascontiguousarray(battr))
        place(blob, "amfT", np.ascontiguousarray(amfT))
        place(blob, "slot", sloti)
        place(blob, "xi_i", _wrap16(gi_dst.astype(np.int16)))
        place(blob, "xja_i", _wrap16(idxa))
        place(blob, "xjb_i", _wrap16(idxb))
        place(blob, "nid_i", nid)
        place(blob, "nattr", nattr)
        place(blob, "anfT", anfT)
        place(blob, "bias", biases)
        in_maps.append({"blob": blob})
    return in_maps, slot2node, T_B, E_pad


# --------------------------------------------------------------------------
# Device kernel builder
# --------------------------------------------------------------------------

def _build(T_B):
    win_cap = T_B * P
    E_pad = NWIN * win_cap
    ntiles = NWIN * T_B
    offs, blob_bytes = _layout(T_B)

    nc = bacc.Bacc("TRN2", target_bir_lowering=False, debug=False,
                   num_devices=NCORES)

    d_blob = nc.dram_tensor("blob", [blob_bytes], U8, kind="ExternalInput")
    d_out = nc.dram_tensor("out", [NODE_SLOTS, D], F16, kind="ExternalOutput")

    def bslice(name, nbytes, dt):
        o = offs[name]
        isz = mybir.dt.size(dt)
        return d_blob.ap()[o:o + nbytes].bitcast(dt)

    def b2d(name, rows, cols, dt):
        isz = mybir.dt.size(dt)
        return bslice(name, rows * cols * isz, dt).rearrange(
            "(r c) -> r c", c=cols)

    # raw Internal DRAM (dma_gather source must be Internal Local)
    x_loc = nc.dram_tensor("x_loc", [NODE_SLOTS, D], BF16)
    x_all = nc.dram_tensor("x_all", [VTOT, D], BF16)
    w_loc = nc.dram_tensor("w_loc", [WSH, KO], BF16)
    w_all = nc.dram_tensor("w_all", [WROWS, KO], BF16)

    mult = mybir.AluOpType.mult
    add = mybir.AluOpType.add
    iseq = mybir.AluOpType.is_equal
    silu = mybir.ActivationFunctionType.Silu

    from concourse.masks import make_identity

    with tile.TileContext(nc) as tc:
        with (
            tc.tile_pool(name="const", bufs=1) as cpool,
            tc.tile_pool(name="gat", bufs=2) as gpool,
            tc.tile_pool(name="ain", bufs=2) as apool,
            tc.tile_pool(name="work", bufs=3) as wpool,
            tc.tile_pool(name="cps", bufs=2, space="PSUM") as cps,
            tc.tile_pool(name="trps", bufs=2, space="PSUM") as trps,
            tc.tile_pool(name="aggps", bufs=1, space="PSUM") as aggps,
        ):
            # ---- phase 0: assemble replicated x and weight tables ----
            nc.sync.dma_start(x_loc.ap(), b2d("xsh", NODE_SLOTS, D, BF16))
            nc.sync.dma_start(w_loc.ap(), b2d("wsh", WSH, KO, BF16))
            nc.gpsimd.collective_compute(
                "AllGather", mybir.AluOpType.bypass,
                replica_groups=[list(range(NCORES))],
                ins=[x_loc.ap().opt()],
                outs=[x_all.ap().opt()],
            )
            nc.gpsimd.collective_compute(
                "AllGather", mybir.AluOpType.bypass,
                replica_groups=[list(range(NCORES))],
                ins=[w_loc.ap().opt()],
                outs=[w_all.ap().opt()],
            )

            # idx slabs, replicated 8x down the partitions for the 8 Q7 cores
            xi_s = cpool.tile([P, E_pad // 16], I16, tag="xi_s", name="xi_s")
            xja_s = cpool.tile([P, E_pad // 16], I16, tag="xja_s", name="xja_s")
            xjb_s = cpool.tile([P, E_pad // 16], I16, tag="xjb_s", name="xjb_s")
            nid_s = cpool.tile([P, NODE_SLOTS // 16], I16, tag="nid_s", name="nid_s")
            for k in range(8):
                sl = slice(16 * k, 16 * (k + 1))
                nc.sync.dma_start(xi_s[sl, :], b2d("xi_i", 16, E_pad // 16, I16))
                nc.sync.dma_start(xja_s[sl, :], b2d("xja_i", 16, E_pad // 16, I16))
                nc.sync.dma_start(xjb_s[sl, :], b2d("xjb_i", 16, E_pad // 16, I16))
                nc.sync.dma_start(nid_s[sl, :], b2d("nid_i", 16, NODE_SLOTS // 16, I16))

            slot_i = cpool.tile([P, ntiles], I16, tag="slot_i", name="slot_i")
            nc.sync.dma_start(slot_i[:], b2d("slot", P, ntiles, I16))
            slot_s = cpool.tile([P, ntiles], F32, tag="slot_s", name="slot_s")
            nc.vector.tensor_copy(slot_s[:], slot_i[:])

            # ---- constants resident in SBUF ----
            ident = cpool.tile([P, P], BF16, tag="ident", name="ident")
            make_identity(nc, ident[:])

            iota_i = cpool.tile([P, SLOTS], I32, tag="iota_i", name="iota_i")
            nc.gpsimd.iota(iota_i[:], pattern=[[1, SLOTS]], channel_multiplier=0)
            iota_f = cpool.tile([P, SLOTS], F32, tag="iota_f", name="iota_f")
            nc.vector.tensor_copy(iota_f[:], iota_i[:])

            # biases: [4, D] f32 in blob, broadcast to [P, D] via stride-0 DMA
            btile = [cpool.tile([P, D], F32, tag=f"b{i}r", name=f"b{i}r")
                     for i in range(4)]
            for i in range(4):
                row = bslice("bias", 4 * D * 4, F32)[i * D:(i + 1) * D]
                nc.sync.dma_start(
                    btile[i][:], row.rearrange("(o d) -> o d", o=1)
                    .to_broadcast([P, D]))

            # node-side attr slabs
            na_s = cpool.tile([P, NODE_SLOTS // P * A], F32, tag="na_s", name="na_s")
            na_bf = cpool.tile([P, NODE_SLOTS // P * A], BF16, tag="na_bf", name="na_bf")
            nc.sync.dma_start(
                na_bf[:].rearrange("p (t a) -> p t a", a=A),
                b2d("nattr", NODE_SLOTS, A, BF16).rearrange(
                    "(t p) a -> p t a", p=P))
            nc.vector.tensor_copy(na_s[:], na_bf[:])
            anf_s = cpool.tile([AM, NODE_SLOTS], BF16, tag="anf_s", name="anf_s")
            nc.sync.dma_start(anf_s[:], b2d("anfT", AM, NODE_SLOTS, BF16))

            aggT = cpool.tile([P, NODE_SLOTS], BF16, tag="aggT", name="aggT")

            # weight tiles from the allgathered table
            w1c = [cpool.tile([P, KO], BF16, tag="w1c0", name="w1c0"),
                   cpool.tile([P, KO], BF16, tag="w1c1", name="w1c1"),
                   cpool.tile([AM, KO], BF16, tag="w1c2", name="w1c2")]
            w2c = cpool.tile([P, KO], BF16, tag="w2c", name="w2c")
            w3c = [cpool.tile([P, KO], BF16, tag="w3c0", name="w3c0"),
                   cpool.tile([P, KO], BF16, tag="w3c1", name="w3c1"),
                   cpool.tile([AM, KO], BF16, tag="w3c2", name="w3c2")]
            w4c = cpool.tile([P, KO], BF16, tag="w4c", name="w4c")

            # gathers/weight loads below need x_all/w_all complete
            tc.strict_bb_all_engine_barrier()

            r0 = 0
            for tiles, rows in ((w1c, (P, P, AM)), ((w2c,), (P,)),
                                (w3c, (P, P, AM)), ((w4c,), (P,))):
                for tl, nr in zip(tiles, rows):
                    nc.sync.dma_start(tl[:], w_all.ap()[r0:r0 + nr, :])
                    r0 += nr

            # ---- helpers ----
            def gather_T(dst_ap, src_ap, idx_slab, i0, n):
                nc.gpsimd.dma_gather(
                    out_ap=dst_ap.rearrange("p (o f) -> p o f", o=1),
                    in_ap=src_ap,
                    idxs_ap=idx_slab[:, i0 // 16:(i0 + n) // 16],
                    num_idxs=n, num_idxs_reg=n, elem_size=P, transpose=True)

            def tp_layer(chunks, wchunks, bt, bias_rep, out_tile, do_silu):
                cpsum = cps.tile([P, KO], F32, tag="c", name="c")
                nch = len(chunks)
                for ci in range(nch):
                    for h in range(2):
                        nc.tensor.matmul(
                            cpsum[:, h * 512:(h + 1) * 512],
                            lhsT=chunks[ci],
                            rhs=wchunks[ci][:, h * 512:(h + 1) * 512],
                            start=(ci == 0),
                            stop=(ci == nch - 1),
                        )
                acc = wpool.tile([P, D], F32, tag="acc", name="acc")
                nc.vector.scalar_tensor_tensor(
                    acc[:], cpsum[:, 0:D], bt[:, 0:1], bias_rep[:], mult, add)
                for k in range(1, A):
                    nc.vector.scalar_tensor_tensor(
                        acc[:], cpsum[:, k * D:(k + 1) * D], bt[:, k:k + 1],
                        acc[:], mult, add)
                if do_silu:
                    nc.scalar.activation(out_tile[:], acc[:], silu)
                else:
                    nc.vector.tensor_copy(out_tile[:], acc[:])

            def transpose_to(src_bf16):
                tps = trps.tile([P, P], BF16, tag="tr", name="tr")
                nc.tensor.transpose(tps[:], src_bf16[:], ident[:])
                dst = wpool.tile([P, P], BF16, tag="mT", name="mT")
                nc.vector.tensor_copy(dst[:], tps[:])
                return dst

            xa_half = x_all.ap()[0:HALF, :]
            xb_half = x_all.ap()[HALF:VTOT, :]

            # ---- edge phase ----
            for w in range(NWIN):
                e0 = w * win_cap
                xiw = gpool.tile([P, win_cap], BF16, tag="xiw", name="xiw")
                xjw = gpool.tile([P, win_cap], BF16, tag="xjw", name="xjw")
                xjb = gpool.tile([P, win_cap], BF16, tag="xjb", name="xjb")
                for c0 in range(0, win_cap, GCH):
                    n = min(GCH, win_cap - c0)
                    gather_T(xiw[:, c0:c0 + n], x_loc.ap(), xi_s, e0 + c0, n)
                    gather_T(xjw[:, c0:c0 + n], xa_half, xja_s, e0 + c0, n)
                    gather_T(xjb[:, c0:c0 + n], xb_half, xjb_s, e0 + c0, n)
                nc.vector.tensor_tensor(xjw[:], xjw[:], xjb[:], add)

                amfw = apool.tile([AM, win_cap], BF16, tag="amfw", name="amfw")
                nc.sync.dma_start(
                    amfw[:],
                    bslice("amfT", AM * E_pad * 2, BF16)
                    .rearrange("(m e) -> m e", e=E_pad)[:, e0:e0 + win_cap])
                btw_bf = apool.tile([P, T_B * A], BF16, tag="btwb", name="btwb")
                nc.sync.dma_start(
                    btw_bf[:].rearrange("p (t a) -> p t a", a=A),
                    b2d("battr", E_pad, A, BF16)[e0:e0 + win_cap, :]
                    .rearrange("(t p) a -> p t a", p=P))
                btw = apool.tile([P, T_B * A], F32, tag="btw", name="btw")
                nc.vector.tensor_copy(btw[:], btw_bf[:])

                agg_ps = aggps.tile([P, SLOTS], F32, tag="agg", name="agg")
                for t in range(T_B):
                    gt = w * T_B + t
                    bt = btw[:, t * A:(t + 1) * A]

                    m1 = wpool.tile([P, D], BF16, tag="m1", name="m1")
                    tp_layer([xiw[:, t * P:(t + 1) * P],
                              xjw[:, t * P:(t + 1) * P],
                              amfw[:, t * P:(t + 1) * P]],
                             w1c, bt, btile[0], m1, True)
                    m1T = transpose_to(m1)
                    m2 = wpool.tile([P, D], BF16, tag="m2", name="m2")
                    tp_layer([m1T], [w2c], bt, btile[1], m2, True)

                    St = wpool.tile([P, SLOTS], BF16, tag="St", name="St")
                    nc.vector.tensor_scalar(
                        St[:], iota_f[:], slot_s[:, gt:gt + 1], None, iseq)
                    nc.tensor.matmul(
                        agg_ps[:],
                        lhsT=m2[:],
                        rhs=St[:],
                        start=(t == 0),
                        stop=(t == T_B - 1),
                    )
                nc.vector.tensor_copy(
                    aggT[:, w * SLOTS:(w + 1) * SLOTS], agg_ps[:])

            # ---- node phase ----
            xT = cpool.tile([P, NODE_SLOTS], BF16, tag="xT", name="xT")
            for c0 in range(0, NODE_SLOTS, GCH):
                n = min(GCH, NODE_SLOTS - c0)
                gather_T(xT[:, c0:c0 + n], x_loc.ap(), nid_s, c0, n)

            nnt = NODE_SLOTS // P  # 50
            for t in range(nnt):
                na = na_s[:, t * A:(t + 1) * A]
                u = wpool.tile([P, D], BF16, tag="m1", name="m1")
                tp_layer([xT[:, t * P:(t + 1) * P],
                          aggT[:, t * P:(t + 1) * P],
                          anf_s[:, t * P:(t + 1) * P]],
                         w3c, na, btile[2], u, True)
                uT = transpose_to(u)
                out_t = wpool.tile([P, D], F16, tag="outt", name="outt")
                tp_layer([uT], [w4c], na, btile[3], out_t, False)
                nc.sync.dma_start(d_out.ap()[t * P:(t + 1) * P, :], out_t[:])

    nc.compile()
    return nc


# --------------------------------------------------------------------------
# Cached PJRT execution (explicit sharded device_put + cached jit)
# --------------------------------------------------------------------------

def _get_exec(nc):
    key = id(nc)
    if key in _exec_cache:
        return _exec_cache[key]

    import jax
    import jax.numpy as jnp
    from jax.sharding import Mesh, PartitionSpec, NamedSharding
    from jax.experimental.shard_map import shard_map
    from concourse.bass2jax import (
        _bass_exec_p, install_neuronx_cc_hook, partition_id_tensor)

    install_neuronx_cc_hook()

    partition_name = nc.partition_id_tensor.name if nc.partition_id_tensor else None
    in_names, out_names, out_avals = [], [], []
    zero_shapes = []
    for alloc in nc.m.functions[0].allocations:
        if not isinstance(alloc, mybir.MemoryLocationSet):
            continue
        name = alloc.memorylocations[0].name
        if alloc.kind == "ExternalInput":
            if name != partition_name:
                in_names.append(name)
        elif alloc.kind == "ExternalOutput":
            out_names.append(name)
            shape = tuple(alloc.tensor_shape)
            dtype = mybir.dt.np(alloc.dtype)
            out_avals.append(jax.core.ShapedArray(shape, dtype))
            zero_shapes.append(((NCORES * shape[0], *shape[1:]), dtype))
    n_params = len(in_names)
    n_outs = len(out_avals)
    all_in = in_names + out_names
    if partition_name is not None:
        all_in.append(partition_name)

    def _body(*args):
        operands = list(args)
        if partition_name is not None:
            operands.append(partition_id_tensor())
        outs = _bass_exec_p.bind(
            *operands,
            out_avals=tuple(out_avals),
            in_names=tuple(all_in),
            out_names=tuple(out_names),
            lowering_input_output_aliases=(),
            sim_require_finite=True,
            sim_require_nnan=True,
            nc=nc,
        )
        return tuple(outs)

    devices = jax.devices()[:NCORES]
    mesh = Mesh(np.asarray(devices), ("core",))
    sharding = NamedSharding(mesh, PartitionSpec("core"))
    donate = tuple(range(n_params, n_params + n_outs))
    sharded = jax.jit(
        shard_map(_body, mesh=mesh,
                  in_specs=(PartitionSpec("core"),) * (n_params + n_outs),
                  out_specs=(PartitionSpec("core"),) * n_outs,
                  check_rep=False),
        donate_argnums=donate,
        keep_unused=True,
    )

    # donated output buffers are generated on-device (no host transfer)
    def _mkzeros():
        return tuple(jnp.zeros(s, d) for s, d in zero_shapes)
    zeros_fn = jax.jit(_mkzeros, out_shardings=(sharding,) * n_outs)

    meta = (in_names, out_names, out_avals, zeros_fn, sharding, sharded)
    _exec_cache[key] = meta
    return meta


def _run_fast(nc, in_maps):
    """One full execution: host concat -> device transfer -> NEFF run ->
    fetch outputs.  Returns a list of per-core {name: np.ndarray}."""
    import jax

    in_names, out_names, out_avals, zeros_fn, sharding, sharded = _get_exec(nc)

    concat_in = [
        np.concatenate([np.asarray(m[nm]) for m in in_maps], axis=0)
        for nm in in_names
    ]
    dev_zero = zeros_fn()
    dev_in = [jax.device_put(a, sharding) for a in concat_in]
    out_arrs = sharded(*dev_in, *dev_zero)
    host_out = [np.asarray(a) for a in out_arrs]
    return [
        {name: host_out[i].reshape(NCORES, *out_avals[i].shape)[c]
         for i, name in enumerate(out_names)}
        for c in range(NCORES)
    ]


# --------------------------------------------------------------------------
# Entry point
# --------------------------------------------------------------------------

def kernel(x, edge_attr, node_attr, additional_message_features,
           additional_node_features, W1, b1, W2, b2, W3, b3, W4, b4,
           edge_index, batch=None):
    in_maps, slot2node, T_B, E_pad = _prepare(
        x, edge_attr, node_attr, additional_message_features,
        additional_node_features, W1, b1, W2, b2, W3, b3, W4, b4, edge_index)

    if T_B not in _nc_cache:
        _nc_cache[T_B] = _build(T_B)
    nc = _nc_cache[T_B]

    results = _run_fast(nc, in_maps)
    kernel.last = (nc, in_maps, results)

    out = np.zeros((N, D), dtype=np.float32)
    for c in range(NCORES):
        oc = results[c]["out"].astype(np.float32)
        mask = slot2node[c] >= 0
        out[slot2node[c][mask]] = oc[mask]
    return out


# revision 10
# speedup vs baseline: 19.2758x; 1.0536x over previous
"""Trainium2 Bass kernel for HSEGNNFlexLayer (GNN message passing).

Strategy (8 NeuronCores, SPMD):
  - Host assigns each node to a (core, window, slot) bin: 8 cores x 25
    windows x 256 slots.  Every edge is routed to the core that owns its
    dst node, so the segment-sum is fully local to each core.
  - x is staged SHARDED (one slot-major [6400, 128] bf16 shard per core)
    and assembled on-device into a replicated slot-major table via an
    AllGather over NeuronLink.  Weights are likewise sharded and
    allgathered.  x_i / x_j are gathered ON DEVICE with
    dma_gather(transpose=True), which lands feature-major tiles directly
    — the host never stages per-edge gathered features.
  - The scatter one-hot S is built on device per tile via
    tensor_scalar(is_equal) against an iota row, from a staged slot id.
  - Message layers: c = a @ Wflat with edges on PSUM partitions,
    attr-weighted k-sum via scalar_tensor_tensor chains, Silu on ScalarE.
  - Scatter-add: one-hot S matmul accumulating into a per-window PSUM
    bank; flushed to an SBUF-resident transposed aggregate.
  - Node update layers run the same pipeline over the 6400 node slots.

All per-core inputs are packed into ONE contiguous byte blob so each
call performs a single large host->device transfer (the axon tunnel has
high per-array overhead).  The donated zero output buffers are generated
on-device.  Execution goes through a cached jax.jit of the bass_exec
shard_map; every call still moves all inputs host->device and all
outputs device->host.
"""

import numpy as np
import ml_dtypes

import concourse.bass as bass
import concourse.mybir as mybir
import concourse.tile as tile
from concourse import bacc

# Problem constants (hardcoded per contest contract)
N, E, D, A, AM = 50000, 500000, 128, 8, 3
MIN_DIM = 2 * D + AM  # 259
UIN_DIM = D + D + AM  # 259
NCORES = 8
P = 128
KO = A * D  # 1024 = flattened (k, o) output columns per TP layer
SLOTS = 256  # node slots per window (one PSUM bank of f32)
NWIN = 25
NODE_SLOTS = NWIN * SLOTS  # 6400 per core
VTOT = NCORES * NODE_SLOTS  # 51200 rows in the allgathered table
HALF = VTOT // 2
ZROW = 24 * SLOTS  # reserved always-zero slot (window 24, slot 0)
GCH = 512  # dma_gather chunk (hardware transpose-gather limit is ~896)
WROWS = 2 * (MIN_DIM + D) + 2  # 776 packed weight rows (pad to 8*97)
WSH = WROWS // NCORES  # 97 weight rows staged per core
BF16 = mybir.dt.bfloat16
F16 = mybir.dt.float16
F32 = mybir.dt.float32
I16 = mybir.dt.int16
I32 = mybir.dt.int32
U8 = mybir.dt.uint8
NPBF16 = ml_dtypes.bfloat16

_nc_cache = {}
_exec_cache = {}
_concat_cache = {}


def _layout(T_B):
    """Byte offsets of each logical tensor inside the per-core blob."""
    win_cap = T_B * P
    E_pad = NWIN * win_cap
    ntiles = NWIN * T_B
    fields = [
        ("xsh", NODE_SLOTS * D * 2),
        ("wsh", WSH * KO * 2),
        ("battr", E_pad * A * 2),
        ("amfT", AM * E_pad * 2),
        ("slot", P * ntiles * 2),
        ("xi_i", E_pad * 2),
        ("xja_i", E_pad * 2),
        ("xjb_i", E_pad * 2),
        ("nid_i", NODE_SLOTS * 2),
        ("nattr", NODE_SLOTS * A * 2),
        ("anfT", AM * NODE_SLOTS * 2),
        ("bias", 4 * D * 4),
    ]
    offs, o = {}, 0
    for name, nbytes in fields:
        offs[name] = o
        o += (nbytes + 3) & ~3  # 4-byte align
    return offs, o


# --------------------------------------------------------------------------
# Host-side preparation
# --------------------------------------------------------------------------

def _assign_nodes(dst):
    """Greedy-pack nodes into NCORES*NWIN bins (<=SLOTS nodes each),
    balancing per-bin edge counts.  Slot 0 of window NWIN-1 on every core
    is reserved (stays zero) so the gather tables have a known zero row.
    Returns (node2bin, node2slot)."""
    import heapq

    counts = np.bincount(dst, minlength=N)
    order = np.argsort(-counts, kind="stable")
    nbins = NCORES * NWIN
    node2bin = np.empty(N, dtype=np.int32)
    node2slot = np.empty(N, dtype=np.int32)
    bin_nodes = np.zeros(nbins, dtype=np.int32)
    for c in range(NCORES):
        bin_nodes[c * NWIN + (NWIN - 1)] = 1  # reserve the zero row
    heap = [(0, b) for b in range(nbins)]
    heapq.heapify(heap)
    for n in order:
        while True:
            c, b = heapq.heappop(heap)
            if bin_nodes[b] < SLOTS:
                break
        node2bin[n] = b
        node2slot[n] = bin_nodes[b]
        bin_nodes[b] += 1
        heapq.heappush(heap, (c + int(counts[n]), b))
    return node2bin, node2slot


def _wrap16(a):
    """Pack an idx vector into the SWDGE [16, n/16] layout (replication to
    [128, n/16] happens on device)."""
    return np.ascontiguousarray(a.reshape(-1, 16).T).astype(np.int16)


def _prepare(x, edge_attr, node_attr, amf, anf, W1, b1, W2, b2, W3, b3, W4, b4,
             edge_index):
    x = np.asarray(x, dtype=np.float32)
    edge_attr = np.asarray(edge_attr, dtype=np.float32)
    node_attr = np.asarray(node_attr, dtype=np.float32)
    amf = np.asarray(amf, dtype=np.float32)
    anf = np.asarray(anf, dtype=np.float32)
    src = np.asarray(edge_index[0]).astype(np.int32)
    dst = np.asarray(edge_index[1]).astype(np.int32)

    node2bin, node2slot = _assign_nodes(dst)
    node_core = node2bin // NWIN
    node_win = node2bin % NWIN
    node_gslot = node_win * SLOTS + node2slot  # slot within core [0, NODE_SLOTS)

    e_bin = node2bin[dst]  # bin (core*NWIN + win) of each edge

    e_order = np.argsort(e_bin, kind="stable")
    e_bin_sorted = e_bin[e_order]
    bin_cnt = np.bincount(e_bin_sorted, minlength=NCORES * NWIN)
    T_B = int(np.ceil(bin_cnt.max() / P))
    win_cap = T_B * P
    E_pad = NWIN * win_cap
    ntiles = NWIN * T_B

    bin_starts = np.zeros(NCORES * NWIN + 1, dtype=np.int64)
    np.cumsum(bin_cnt, out=bin_starts[1:])

    offs_in_bin = np.arange(len(e_order)) - bin_starts[e_bin_sorted]
    pos = (e_bin_sorted % NWIN) * win_cap + offs_in_bin  # position within core
    core_of_edge = e_bin_sorted // NWIN

    # Per-core packed edge arrays (padded entries: eid -1)
    ew_src = np.zeros((NCORES, E_pad), dtype=np.int32)
    ew_dst = np.zeros((NCORES, E_pad), dtype=np.int32)
    ew_eid = np.full((NCORES, E_pad), -1, dtype=np.int64)
    ew_src[core_of_edge, pos] = src[e_order]
    ew_dst[core_of_edge, pos] = dst[e_order]
    ew_eid[core_of_edge, pos] = e_order

    # Packed weight block [WROWS, KO] bf16: w1, w2, w3, w4 stacked (k-major
    # flattened columns Wf[i, k*D + o] = W[i, k, o]); sharded across cores.
    wpack = np.zeros((WROWS, KO), dtype=NPBF16)
    r = 0
    for W in (W1, W2, W3, W4):
        Wf = np.asarray(W, np.float32).reshape(-1, KO)
        wpack[r:r + Wf.shape[0]] = Wf.astype(NPBF16)
        r += Wf.shape[0]
    biases = np.stack([np.asarray(b, np.float32) for b in (b1, b2, b3, b4)])

    g_all = node_core.astype(np.int64) * NODE_SLOTS + node_gslot

    nid = _wrap16(np.arange(NODE_SLOTS, dtype=np.int16))

    offs, blob_bytes = _layout(T_B)
    full_blob = np.zeros((NCORES, blob_bytes), dtype=np.uint8)

    def place(blob, name, arr):
        raw = arr.tobytes()
        blob[offs[name]:offs[name] + len(raw)] = np.frombuffer(raw, np.uint8)

    in_maps = []
    slot2node = np.full((NCORES, NODE_SLOTS), -1, dtype=np.int64)
    for c in range(NCORES):
        s = ew_src[c]
        d = ew_dst[c]
        eid = ew_eid[c]
        valid = eid >= 0
        ev = np.where(valid, eid, 0)

        gi_dst = node_gslot[d].astype(np.int32)
        gi_dst[~valid] = ZROW
        g_src = g_all[s]
        idxa = np.where(valid & (g_src < HALF), g_src, ZROW).astype(np.int16)
        idxb = np.where(valid & (g_src >= HALF), g_src - HALF, ZROW).astype(np.int16)

        # slot of each edge within its window, [128, ntiles] i16; -1 padding
        sloti = np.where(valid, node2slot[d], -1).astype(np.int16)
        sloti = np.ascontiguousarray(sloti.reshape(ntiles, P).T)

        battr = edge_attr[ev].astype(NPBF16)
        battr[~valid] = 0
        amfT = amf[ev].T.astype(NPBF16)
        amfT[:, ~valid] = 0

        nodes_c = np.nonzero(node_core == c)[0]
        gs = node_gslot[nodes_c]
        slot2node[c, gs] = nodes_c
        xsh = np.zeros((NODE_SLOTS, D), dtype=NPBF16)
        xsh[gs] = x[nodes_c].astype(NPBF16)
        anfT = np.zeros((AM, NODE_SLOTS), dtype=NPBF16)
        anfT[:, gs] = anf[nodes_c].T.astype(NPBF16)
        nattr = np.zeros((NODE_SLOTS, A), dtype=NPBF16)
        nattr[gs] = node_attr[nodes_c].astype(NPBF16)

        blob = full_blob[c]
        place(blob, "xsh", xsh)
        place(blob, "wsh", np.ascontiguousarray(wpack[c * WSH:(c + 1) * WSH]))
        place(blob, "battr", np.ascontiguousarray(battr))
        place(blob, "amfT", np.ascontiguousarray(amfT))
        place(blob, "slot", sloti)
        place(blob, "xi_i", _wrap16(gi_dst.astype(np.int16)))
        place(blob, "xja_i", _wrap16(idxa))
        place(blob, "xjb_i", _wrap16(idxb))
        place(blob, "nid_i", nid)
        place(blob, "nattr", nattr)
        place(blob, "anfT", anfT)
        place(blob, "bias", biases)
        in_maps.append({"blob": blob})
    _concat_cache[id(in_maps)] = {"blob": full_blob.reshape(-1)}
    return in_maps, slot2node, T_B, E_pad


# --------------------------------------------------------------------------
# Device kernel builder
# --------------------------------------------------------------------------

def _build(T_B):
    win_cap = T_B * P
    E_pad = NWIN * win_cap
    ntiles = NWIN * T_B
    offs, blob_bytes = _layout(T_B)

    nc = bacc.Bacc("TRN2", target_bir_lowering=False, debug=False,
                   num_devices=NCORES)

    d_blob = nc.dram_tensor("blob", [blob_bytes], U8, kind="ExternalInput")
    d_out = nc.dram_tensor("out", [NODE_SLOTS, D], F16, kind="ExternalOutput")

    def bslice(name, nbytes, dt):
        o = offs[name]
        isz = mybir.dt.size(dt)
        return d_blob.ap()[o:o + nbytes].bitcast(dt)

    def b2d(name, rows, cols, dt):
        isz = mybir.dt.size(dt)
        return bslice(name, rows * cols * isz, dt).rearrange(
            "(r c) -> r c", c=cols)

    # raw Internal DRAM (dma_gather source must be Internal Local)
    x_loc = nc.dram_tensor("x_loc", [NODE_SLOTS, D], BF16)
    x_all = nc.dram_tensor("x_all", [VTOT, D], BF16)
    w_loc = nc.dram_tensor("w_loc", [WSH, KO], BF16)
    w_all = nc.dram_tensor("w_all", [WROWS, KO], BF16)

    mult = mybir.AluOpType.mult
    add = mybir.AluOpType.add
    iseq = mybir.AluOpType.is_equal
    silu = mybir.ActivationFunctionType.Silu

    from concourse.masks import make_identity

    with tile.TileContext(nc) as tc:
        with (
            tc.tile_pool(name="const", bufs=1) as cpool,
            tc.tile_pool(name="gat", bufs=2) as gpool,
            tc.tile_pool(name="ain", bufs=2) as apool,
            tc.tile_pool(name="work", bufs=3) as wpool,
            tc.tile_pool(name="cps", bufs=2, space="PSUM") as cps,
            tc.tile_pool(name="trps", bufs=2, space="PSUM") as trps,
            tc.tile_pool(name="aggps", bufs=1, space="PSUM") as aggps,
        ):
            # ---- phase 0: assemble replicated x and weight tables ----
            nc.sync.dma_start(x_loc.ap(), b2d("xsh", NODE_SLOTS, D, BF16))
            nc.sync.dma_start(w_loc.ap(), b2d("wsh", WSH, KO, BF16))
            nc.gpsimd.collective_compute(
                "AllGather", mybir.AluOpType.bypass,
                replica_groups=[list(range(NCORES))],
                ins=[x_loc.ap().opt()],
                outs=[x_all.ap().opt()],
            )
            nc.gpsimd.collective_compute(
                "AllGather", mybir.AluOpType.bypass,
                replica_groups=[list(range(NCORES))],
                ins=[w_loc.ap().opt()],
                outs=[w_all.ap().opt()],
            )

            # idx slabs, replicated 8x down the partitions for the 8 Q7 cores
            xi_s = cpool.tile([P, E_pad // 16], I16, tag="xi_s", name="xi_s")
            xja_s = cpool.tile([P, E_pad // 16], I16, tag="xja_s", name="xja_s")
            xjb_s = cpool.tile([P, E_pad // 16], I16, tag="xjb_s", name="xjb_s")
            nid_s = cpool.tile([P, NODE_SLOTS // 16], I16, tag="nid_s", name="nid_s")
            for k in range(8):
                sl = slice(16 * k, 16 * (k + 1))
                nc.sync.dma_start(xi_s[sl, :], b2d("xi_i", 16, E_pad // 16, I16))
                nc.sync.dma_start(xja_s[sl, :], b2d("xja_i", 16, E_pad // 16, I16))
                nc.sync.dma_start(xjb_s[sl, :], b2d("xjb_i", 16, E_pad // 16, I16))
                nc.sync.dma_start(nid_s[sl, :], b2d("nid_i", 16, NODE_SLOTS // 16, I16))

            slot_i = cpool.tile([P, ntiles], I16, tag="slot_i", name="slot_i")
            nc.sync.dma_start(slot_i[:], b2d("slot", P, ntiles, I16))
            slot_s = cpool.tile([P, ntiles], F32, tag="slot_s", name="slot_s")
            nc.vector.tensor_copy(slot_s[:], slot_i[:])

            # ---- constants resident in SBUF ----
            ident = cpool.tile([P, P], BF16, tag="ident", name="ident")
            make_identity(nc, ident[:])

            iota_i = cpool.tile([P, SLOTS], I32, tag="iota_i", name="iota_i")
            nc.gpsimd.iota(iota_i[:], pattern=[[1, SLOTS]], channel_multiplier=0)
            iota_f = cpool.tile([P, SLOTS], F32, tag="iota_f", name="iota_f")
            nc.vector.tensor_copy(iota_f[:], iota_i[:])

            # biases: [4, D] f32 in blob, broadcast to [P, D] via stride-0 DMA
            btile = [cpool.tile([P, D], F32, tag=f"b{i}r", name=f"b{i}r")
                     for i in range(4)]
            for i in range(4):
                row = bslice("bias", 4 * D * 4, F32)[i * D:(i + 1) * D]
                nc.sync.dma_start(
                    btile[i][:], row.rearrange("(o d) -> o d", o=1)
                    .to_broadcast([P, D]))

            # node-side attr slabs
            na_s = cpool.tile([P, NODE_SLOTS // P * A], F32, tag="na_s", name="na_s")
            na_bf = cpool.tile([P, NODE_SLOTS // P * A], BF16, tag="na_bf", name="na_bf")
            nc.sync.dma_start(
                na_bf[:].rearrange("p (t a) -> p t a", a=A),
                b2d("nattr", NODE_SLOTS, A, BF16).rearrange(
                    "(t p) a -> p t a", p=P))
            nc.vector.tensor_copy(na_s[:], na_bf[:])
            anf_s = cpool.tile([AM, NODE_SLOTS], BF16, tag="anf_s", name="anf_s")
            nc.sync.dma_start(anf_s[:], b2d("anfT", AM, NODE_SLOTS, BF16))

            aggT = cpool.tile([P, NODE_SLOTS], BF16, tag="aggT", name="aggT")

            # weight tiles from the allgathered table
            w1c = [cpool.tile([P, KO], BF16, tag="w1c0", name="w1c0"),
                   cpool.tile([P, KO], BF16, tag="w1c1", name="w1c1"),
                   cpool.tile([AM, KO], BF16, tag="w1c2", name="w1c2")]
            w2c = cpool.tile([P, KO], BF16, tag="w2c", name="w2c")
            w3c = [cpool.tile([P, KO], BF16, tag="w3c0", name="w3c0"),
                   cpool.tile([P, KO], BF16, tag="w3c1", name="w3c1"),
                   cpool.tile([AM, KO], BF16, tag="w3c2", name="w3c2")]
            w4c = cpool.tile([P, KO], BF16, tag="w4c", name="w4c")

            # gathers/weight loads below need x_all/w_all complete
            tc.strict_bb_all_engine_barrier()

            r0 = 0
            for tiles, rows in ((w1c, (P, P, AM)), ((w2c,), (P,)),
                                (w3c, (P, P, AM)), ((w4c,), (P,))):
                for tl, nr in zip(tiles, rows):
                    nc.sync.dma_start(tl[:], w_all.ap()[r0:r0 + nr, :])
                    r0 += nr

            # ---- helpers ----
            def gather_T(dst_ap, src_ap, idx_slab, i0, n):
                nc.gpsimd.dma_gather(
                    out_ap=dst_ap.rearrange("p (o f) -> p o f", o=1),
                    in_ap=src_ap,
                    idxs_ap=idx_slab[:, i0 // 16:(i0 + n) // 16],
                    num_idxs=n, num_idxs_reg=n, elem_size=P, transpose=True)

            def tp_layer(chunks, wchunks, bt, bias_rep, out_tile, do_silu):
                cpsum = cps.tile([P, KO], F32, tag="c", name="c")
                nch = len(chunks)
                for ci in range(nch):
                    for h in range(2):
                        nc.tensor.matmul(
                            cpsum[:, h * 512:(h + 1) * 512],
                            lhsT=chunks[ci],
                            rhs=wchunks[ci][:, h * 512:(h + 1) * 512],
                            start=(ci == 0),
                            stop=(ci == nch - 1),
                        )
                acc = wpool.tile([P, D], F32, tag="acc", name="acc")
                nc.vector.scalar_tensor_tensor(
                    acc[:], cpsum[:, 0:D], bt[:, 0:1], bias_rep[:], mult, add)
                for k in range(1, A):
                    nc.vector.scalar_tensor_tensor(
                        acc[:], cpsum[:, k * D:(k + 1) * D], bt[:, k:k + 1],
                        acc[:], mult, add)
                if do_silu:
                    nc.scalar.activation(out_tile[:], acc[:], silu)
                else:
                    nc.vector.tensor_copy(out_tile[:], acc[:])

            def transpose_to(src_bf16):
                tps = trps.tile([P, P], BF16, tag="tr", name="tr")
                nc.tensor.transpose(tps[:], src_bf16[:], ident[:])
                dst = wpool.tile([P, P], BF16, tag="mT", name="mT")
                nc.vector.tensor_copy(dst[:], tps[:])
                return dst

            xa_half = x_all.ap()[0:HALF, :]
            xb_half = x_all.ap()[HALF:VTOT, :]

            # ---- edge phase ----
            for w in range(NWIN):
                e0 = w * win_cap
                xiw = gpool.tile([P, win_cap], BF16, tag="xiw", name="xiw")
                xjw = gpool.tile([P, win_cap], BF16, tag="xjw", name="xjw")
                xjb = gpool.tile([P, win_cap], BF16, tag="xjb", name="xjb")
                for c0 in range(0, win_cap, GCH):
                    n = min(GCH, win_cap - c0)
                    gather_T(xiw[:, c0:c0 + n], x_loc.ap(), xi_s, e0 + c0, n)
                    gather_T(xjw[:, c0:c0 + n], xa_half, xja_s, e0 + c0, n)
                    gather_T(xjb[:, c0:c0 + n], xb_half, xjb_s, e0 + c0, n)
                nc.vector.tensor_tensor(xjw[:], xjw[:], xjb[:], add)

                amfw = apool.tile([AM, win_cap], BF16, tag="amfw", name="amfw")
                nc.sync.dma_start(
                    amfw[:],
                    bslice("amfT", AM * E_pad * 2, BF16)
                    .rearrange("(m e) -> m e", e=E_pad)[:, e0:e0 + win_cap])
                btw_bf = apool.tile([P, T_B * A], BF16, tag="btwb", name="btwb")
                nc.sync.dma_start(
                    btw_bf[:].rearrange("p (t a) -> p t a", a=A),
                    b2d("battr", E_pad, A, BF16)[e0:e0 + win_cap, :]
                    .rearrange("(t p) a -> p t a", p=P))
                btw = apool.tile([P, T_B * A], F32, tag="btw", name="btw")
                nc.vector.tensor_copy(btw[:], btw_bf[:])

                agg_ps = aggps.tile([P, SLOTS], F32, tag="agg", name="agg")
                for t in range(T_B):
                    gt = w * T_B + t
                    bt = btw[:, t * A:(t + 1) * A]

                    m1 = wpool.tile([P, D], BF16, tag="m1", name="m1")
                    tp_layer([xiw[:, t * P:(t + 1) * P],
                              xjw[:, t * P:(t + 1) * P],
                              amfw[:, t * P:(t + 1) * P]],
                             w1c, bt, btile[0], m1, True)
                    m1T = transpose_to(m1)
                    m2 = wpool.tile([P, D], BF16, tag="m2", name="m2")
                    tp_layer([m1T], [w2c], bt, btile[1], m2, True)

                    St = wpool.tile([P, SLOTS], BF16, tag="St", name="St")
                    nc.vector.tensor_scalar(
                        St[:], iota_f[:], slot_s[:, gt:gt + 1], None, iseq)
                    nc.tensor.matmul(
                        agg_ps[:],
                        lhsT=m2[:],
                        rhs=St[:],
                        start=(t == 0),
                        stop=(t == T_B - 1),
                    )
                nc.vector.tensor_copy(
                    aggT[:, w * SLOTS:(w + 1) * SLOTS], agg_ps[:])

            # ---- node phase ----
            xT = cpool.tile([P, NODE_SLOTS], BF16, tag="xT", name="xT")
            for c0 in range(0, NODE_SLOTS, GCH):
                n = min(GCH, NODE_SLOTS - c0)
                gather_T(xT[:, c0:c0 + n], x_loc.ap(), nid_s, c0, n)

            nnt = NODE_SLOTS // P  # 50
            for t in range(nnt):
                na = na_s[:, t * A:(t + 1) * A]
                u = wpool.tile([P, D], BF16, tag="m1", name="m1")
                tp_layer([xT[:, t * P:(t + 1) * P],
                          aggT[:, t * P:(t + 1) * P],
                          anf_s[:, t * P:(t + 1) * P]],
                         w3c, na, btile[2], u, True)
                uT = transpose_to(u)
                out_t = wpool.tile([P, D], F16, tag="outt", name="outt")
                tp_layer([uT], [w4c], na, btile[3], out_t, False)
                nc.sync.dma_start(d_out.ap()[t * P:(t + 1) * P, :], out_t[:])

    nc.compile()
    return nc


# --------------------------------------------------------------------------
# Cached PJRT execution (explicit sharded device_put + cached jit)
# --------------------------------------------------------------------------

def _get_exec(nc):
    key = id(nc)
    if key in _exec_cache:
        return _exec_cache[key]

    import jax
    import jax.numpy as jnp
    from jax.sharding import Mesh, PartitionSpec, NamedSharding
    from jax.experimental.shard_map import shard_map
    from concourse.bass2jax import (
        _bass_exec_p, install_neuronx_cc_hook, partition_id_tensor)

    install_neuronx_cc_hook()

    partition_name = nc.partition_id_tensor.name if nc.partition_id_tensor else None
    in_names, out_names, out_avals = [], [], []
    zero_shapes = []
    for alloc in nc.m.functions[0].allocations:
        if not isinstance(alloc, mybir.MemoryLocationSet):
            continue
        name = alloc.memorylocations[0].name
        if alloc.kind == "ExternalInput":
            if name != partition_name:
                in_names.append(name)
        elif alloc.kind == "ExternalOutput":
            out_names.append(name)
            shape = tuple(alloc.tensor_shape)
            dtype = mybir.dt.np(alloc.dtype)
            out_avals.append(jax.core.ShapedArray(shape, dtype))
            zero_shapes.append(((NCORES * shape[0], *shape[1:]), dtype))
    n_params = len(in_names)
    n_outs = len(out_avals)
    all_in = in_names + out_names
    if partition_name is not None:
        all_in.append(partition_name)

    def _body(*args):
        operands = list(args)
        if partition_name is not None:
            operands.append(partition_id_tensor())
        outs = _bass_exec_p.bind(
            *operands,
            out_avals=tuple(out_avals),
            in_names=tuple(all_in),
            out_names=tuple(out_names),
            lowering_input_output_aliases=(),
            sim_require_finite=True,
            sim_require_nnan=True,
            nc=nc,
        )
        return tuple(outs)

    devices = jax.devices()[:NCORES]
    mesh = Mesh(np.asarray(devices), ("core",))
    sharding = NamedSharding(mesh, PartitionSpec("core"))
    donate = tuple(range(n_params, n_params + n_outs))
    sharded = jax.jit(
        shard_map(_body, mesh=mesh,
                  in_specs=(PartitionSpec("core"),) * (n_params + n_outs),
                  out_specs=(PartitionSpec("core"),) * n_outs,
                  check_rep=False),
        donate_argnums=donate,
        keep_unused=True,
    )

    # donated output buffers are generated on-device (no host transfer)
    def _mkzeros():
        return tuple(jnp.zeros(s, d) for s, d in zero_shapes)
    zeros_fn = jax.jit(_mkzeros, out_shardings=(sharding,) * n_outs)

    meta = (in_names, out_names, out_avals, zeros_fn, sharding, sharded)
    _exec_cache[key] = meta
    return meta


def _run_fast(nc, in_maps):
    """One full execution: host concat -> device transfer -> NEFF run ->
    fetch outputs.  Returns a list of per-core {name: np.ndarray}."""
    import jax

    in_names, out_names, out_avals, zeros_fn, sharding, sharded = _get_exec(nc)

    pre = _concat_cache.get(id(in_maps))
    concat_in = [
        pre[nm] if pre is not None and nm in pre else
        np.concatenate([np.asarray(m[nm]) for m in in_maps], axis=0)
        for nm in in_names
    ]
    dev_zero = zeros_fn()
    dev_in = [jax.device_put(a, sharding) for a in concat_in]
    out_arrs = sharded(*dev_in, *dev_zero)
    host_out = [np.asarray(a) for a in out_arrs]
    return [
        {name: host_out[i].reshape(NCORES, *out_avals[i].shape)[c]
         for i, name in enumerate(out_names)}
        for c in range(NCORES)
    ]


# --------------------------------------------------------------------------
# Entry point
# --------------------------------------------------------------------------

def kernel(x, edge_attr, node_attr, additional_message_features,
           additional_node_features, W1, b1, W2, b2, W3, b3, W4, b4,
           edge_index, batch=None):
    in_maps, slot2node, T_B, E_pad = _prepare(
        x, edge_attr, node_attr, additional_message_features,
        additional_node_features, W1, b1, W2, b2, W3, b3, W4, b4, edge_index)

    if T_B not in _nc_cache:
        _nc_cache[T_B] = _build(T_B)
    nc = _nc_cache[T_B]

    results = _run_fast(nc, in_maps)
    kernel.last = (nc, in_maps, results)

    out = np.zeros((N, D), dtype=np.float32)
    for c in range(NCORES):
        oc = results[c]["out"].astype(np.float32)
        mask = slot2node[c] >= 0
        out[slot2node[c][mask]] = oc[mask]
    return out
